# revision 24
# baseline (speedup 1.0000x reference)
"""Trainium2 Bass kernel for nn_DilatedContextAttentionModule (B=8, C=256, 64x64).

Reference, per batch element (N = 64*64 = 4096):
    g   = G xj + g_b 1^T;  th = T xi + t_b 1^T;  phi = P xj + p_b 1^T
    f   = th^T phi / N                      (N x N, linear -- NO softmax)
    y[c,n] = sum_m f[n,m] g[c,m]
    z   = W y + W_b 1^T + xi
    out = BatchNorm2d(z)                    (training-mode batch stats)

Algebraic collapse (Gram-matrix form; exact because f is linear):
    z = A xi + d 1^T,  A = I + E'
    E' = L'(K/N) R + a1 b1^T + wgb b2n^T,  K = xj xj^T  (C x C Gram)
    with host-folded L' = W G, R = P^T T, wgb = W g_b, b1 = T^T p_b,
    ptb = P^T t_b, c1 = p_b.t_b, and runtime vectors from sxj = xj @ 1:
    a1 = L'sxj/N + wgb,  b2n = R^T sxj/N,
    d = L'(K/N)ptb + c1 a1 + (sxj.ptb/N) wgb + W_b.

KEY STRUCTURE (vs the first 57.5us version):
 1. BN statistics computed ANALYTICALLY from a second Gram Kxi = xi xi^T:
        S1/N = A sxi/N + d
        S2/N = diag(A (Kxi/N) A^T) + d*(2 A sxi/N + d)
    so the cross-core ReduceScatter (cost-model floor ~15us) launches
    right after the two C x C Grams + cheap C x C algebra and overlaps
    the entire z-phase, instead of serializing after it.
 2. Both Grams run in fp8e4 with perf_mode=DoubleRowSwInterleave (the
    only DR mode walrus codegen accepts): 0.5 cycles/row with 256-deep
    contraction -> 4.2k PE cycles per Gram instead of 16.5k (f16).
    The fp8 tensor is shipped ONCE in the hardware's interleaved
    weights layout  w8[p,t,m,j,i] = X[m*128+127-j, t*256+i*128+p];
    the matmul rhs reads the SAME tile through a permuted AP view
    (p m j i -> p i m j), which yields Gram columns reversed within
    each 128-chunk; the PSUM->SBUF copy un-reverses with a stride -1
    AP at no cost.  Row sums (sxj, sxi) come from a separate fp8 ones
    rhs accumulated into a spare PSUM column.
    fp8 Gram noise measured end-to-end: rel err ~6.4e-3 (harness gate
    2e-2).
 3. Identity block of A is generated on device (affine_select), not
    shipped; weights DMA is 0.26 MB.
 4. Tail: RS -> ssum -> affine consts (Sqrt on ACT, rest DVE) ->
    out = a*z - b on DVE (f16 4x mode) in a small-first staircase with
    stores alternating SP/ACT queues.
"""

import numpy as np
import ml_dtypes

import concourse.bass as bass
import concourse.bacc as bacc
import concourse.tile as tile
from concourse import mybir
from concourse import bass_utils

B = 8
C = 256
N = 4096          # 64 * 64
NCORES = 8
NCH = 2           # channel chunks of 128
NT2 = 16          # n chunks of 256 (fp8 DoubleRow Gram)
F32 = mybir.dt.float32
F16 = mybir.dt.float16
F8 = mybir.dt.float8e4
BN_EPS = 1e-5

# wmat layout (f16, [128, 2, 516]): per channel-chunk k:
#   [0:256] L'^T rows | [256:512] R rows | [512] ptb | pad
WM_LT = slice(0, 256)
WM_RC = slice(256, 512)
WM_F = 516


def build_kernel(nc, skip_cc: bool = False) -> None:
    f32, f16 = F32, F16
    xjw_d = nc.dram_tensor("xjw8", [128, NT2, 2, 129, 2], F8,
                           kind="ExternalInput").ap()
    xiw_d = nc.dram_tensor("xiw8", [128, NT2, 2, 129, 2], F8,
                           kind="ExternalInput").ap()
    xi_d = nc.dram_tensor("xi", [128, NCH, N], f16, kind="ExternalInput").ap()
    wm_d = nc.dram_tensor("wm", [128, NCH, WM_F], f16, kind="ExternalInput").ap()
    # aux row: [b1 (256) | wgb (256) | c1 (1) | pad]
    aux_d = nc.dram_tensor("aux", [1, 2 * C + 8], f16, kind="ExternalInput").ap()
    # f32 smalls: [gamma | beta | W_b] columns  -> [128, 2, 3]
    sm_d = nc.dram_tensor("sm", [128, NCH, 3], f32, kind="ExternalInput").ap()
    out_d = nc.dram_tensor("out", [C, N], f16, kind="ExternalOutput").ap()

    with tile.TileContext(nc) as tc:
        _body(tc, xjw_d, xiw_d, xi_d, wm_d, aux_d, sm_d, out_d, skip_cc=skip_cc)


def _body(tc, xjw_d, xiw_d, xi_d, wm_d, aux_d, sm_d, out_d,
          skip_cc: bool = False):
    nc = tc.nc
    f32, f16 = F32, F16
    import contextlib
    DR = mybir.MatmulPerfMode.DoubleRowSwInterleave

    with contextlib.ExitStack() as ctx:
        constp = ctx.enter_context(tc.tile_pool(name="const", bufs=1))
        datap = ctx.enter_context(tc.tile_pool(name="data", bufs=1))
        workp = ctx.enter_context(tc.tile_pool(name="work", bufs=4))
        rowsp = ctx.enter_context(tc.tile_pool(name="rows", bufs=2))
        outp = ctx.enter_context(tc.tile_pool(name="out", bufs=1))
        # PSUM budget (8 banks): psacc 2 + psg8 2 + psz 2 + psrow 1 + pscol 1
        psacc = ctx.enter_context(tc.tile_pool(name="ps_acc", bufs=2, space="PSUM"))
        psg8 = ctx.enter_context(tc.tile_pool(name="ps_g8", bufs=2, space="PSUM"))
        psz = ctx.enter_context(tc.tile_pool(name="ps_z", bufs=2, space="PSUM"))
        psrow = ctx.enter_context(tc.tile_pool(name="ps_row", bufs=1, space="PSUM"))
        pscol = ctx.enter_context(tc.tile_pool(name="ps_col", bufs=1, space="PSUM"))
        dramp = ctx.enter_context(tc.tile_pool(name="dram", bufs=2, space="DRAM"))

        # ---- loads, all on the SP queue so the single DMA device ------
        # ---- services them in exactly this priority order -------------
        wm = constp.tile([128, NCH, WM_F], f16, tag="wm")
        nc.sync.dma_start(out=wm, in_=wm_d)
        aux = constp.tile([1, 2 * C + 8], f16, tag="aux")
        nc.sync.dma_start(out=aux, in_=aux_d)
        sm = constp.tile([128, NCH, 3], f32, tag="sm")
        nc.sync.dma_start(out=sm, in_=sm_d)
        xjw = datap.tile([128, NT2, 2, 129, 2], F8, tag="xjw")
        NXJ = 2
        for h in range(NXJ):
            sl = slice(h * (NT2 // NXJ), (h + 1) * (NT2 // NXJ))
            nc.sync.dma_start(out=xjw[:, sl], in_=xjw_d[:, sl])
        xiw = datap.tile([128, NT2, 2, 129, 2], F8, tag="xiw")
        NXI8 = 4
        for h in range(NXI8):
            sl = slice(h * (NT2 // NXI8), (h + 1) * (NT2 // NXI8))
            nc.sync.dma_start(out=xiw[:, sl], in_=xiw_d[:, sl])
        xi_t = datap.tile([128, NCH, N], f16, tag="xi")
        NXI = 4
        for h in range(NXI):
            sl = slice(h * (N // NXI), (h + 1) * (N // NXI))
            nc.sync.dma_start(out=xi_t[:, :, sl], in_=xi_d[:, :, sl])

        eps = constp.tile([128, 1], f32, tag="eps")
        nc.vector.memset(eps, BN_EPS)
        ones1 = constp.tile([128, 1], f16, tag="ones1")
        nc.vector.memset(ones1, 1.0)
        # identity rows, generated on device: ident[:, m, c] = 1 iff
        # c == m*128 + p
        ones256 = constp.tile([128, C], f16, tag="ones256")
        nc.vector.memset(ones256, 1.0)
        ident = constp.tile([128, NCH, C], f16, tag="ident")
        for m in range(NCH):
            nc.gpsimd.affine_select(
                out=ident[:, m, :], in_=ones256, pattern=[[1, C]],
                compare_op=mybir.AluOpType.is_equal, fill=0.0,
                base=-m * 128, channel_multiplier=-1)

        def lt(k, csl=slice(0, C)):
            return wm[:, k, WM_LT][:, csl]

        def rcw(k, csl=slice(0, C)):
            return wm[:, k, WM_RC][:, csl]

        def ptbc(k):
            return wm[:, k, 512:513]

        # ---- PE warm-up: hold the p-state at full clock until the ----
        # ---- first Gram chunk arrives (cold PE runs at 0.65 GHz)  ----
        warm = constp.tile([128, 640], f16, tag="warm")
        nc.gpsimd.memset(warm, 0.0)
        wps = psz.tile([128, 512], f32, tag="zt", name="warm_ps")
        nc.tensor.matmul(wps[:, 0:128], warm[:, 0:128], warm[:, 128:256],
                         start=True, stop=True)
        NWARM = 10
        for w in range(NWARM):
            nc.tensor.matmul(wps, warm[:, 0:128], warm[:, 128:640],
                             start=True, stop=True)

        def gram(ps_tiles, src):
            """fp8 DoubleRowSwInterleave Gram: src is the interleaved
            weights tile [128, NT2, 2, 129, 2] whose j=128 slot holds
            ones, so each matmul's 258-wide output carries the row sums
            at block-relative column 128 inside the SAME accumulation
            group (PSUM zero regions allow one group per bank)."""
            for t in range(NT2):
                rhs = src[:, t].rearrange("p m j i -> p i m j")
                for m in range(NCH):
                    nc.tensor.matmul(
                        ps_tiles[m][:, 0:258], src[:, t, m, 0:128, :], rhs,
                        start=(t == 0), stop=(t == NT2 - 1), perf_mode=DR)

        def gram_copy(ps_tiles, out_tiles, tagp):
            """PSUM -> SBUF f16 with 1/N scale; un-reverses the
            within-chunk column order (psum col = mb*129 + 127 - c,
            row sums at mb*129 + 128); row-sum column copied FIRST so
            consumers of sxn can start early."""
            for m in range(NCH):
                t = workp.tile([128, C + 1], f16, tag=f"{tagp}{m}")
                blk = ps_tiles[m][:, 0:258].rearrange(
                    "p (mb c) -> p mb c", mb=NCH)
                rev = blk[:, :, 0:128][:, :, ::-1]
                if m == 0:
                    nc.scalar.activation(
                        out=t[:, C:C + 1], in_=blk[:, 0, 128:129],
                        func=mybir.ActivationFunctionType.Identity,
                        scale=1.0 / N)
                    nc.scalar.activation(
                        out=t[:, 0:C], in_=rev,
                        func=mybir.ActivationFunctionType.Identity,
                        scale=1.0 / N)
                else:
                    nc.vector.tensor_scalar_mul(
                        t[:, C:C + 1], blk[:, 0, 128:129], 1.0 / N)
                    nc.vector.tensor_scalar_mul(t[:, 0:C], rev, 1.0 / N)
                out_tiles.append(t)

        # ---- phase A: K_aug = [xj|1]^T [xj|1] -> K and sxj ------------
        K_ps = [psacc.tile([128, C + 8], f32, tag="acc", name=f"K_ps{m}")
                for m in range(NCH)]
        gram(K_ps, xjw)
        K_sb = []
        gram_copy(K_ps, K_sb, "K")

        def sxn(k):  # sxj/N column (f16)
            return K_sb[k][:, C:C + 1]

        # ---- phases B/C/D: C x C algebra — high priority so the ------
        # ---- scheduler orders these PE ops ahead of the Kxi Gram -----
        # ---- matmuls (whose xiw DMA waits would head-of-line block ---
        # ---- the in-order PE sequencer) ------------------------------
        hp_ctx = tc.high_priority()
        hp_ctx.__enter__()

        # phase B: runtime rows a1 = L'sxj/N + wgb, b2n = R^T sxj/N
        rows_ps = psrow.tile([1, 2 * C], f32, tag="rows")
        for k in range(NCH):
            nc.tensor.matmul(rows_ps[:, 0:C], sxn(k), lt(k),
                             start=(k == 0), stop=(k == NCH - 1))
        for k in range(NCH):
            nc.tensor.matmul(rows_ps[:, C:2 * C], sxn(k), rcw(k),
                             start=(k == 0), stop=(k == NCH - 1))
        a1row = rowsp.tile([1, C], f16, tag="a1row")
        nc.vector.tensor_add(a1row, rows_ps[:, 0:C], aux[:, C:2 * C])
        b2row = rowsp.tile([1, C], f16, tag="b2row")
        nc.vector.tensor_copy(b2row, rows_ps[:, C:2 * C])

        # phase C: T1 = (K/N) L'^T;  ET = R^T T1 + rank1 + I
        T1_sb = []
        for cb in range(NCH):
            t1_ps = psacc.tile([128, C], f32, tag="acc")
            csl = slice(cb * 128, (cb + 1) * 128)
            for jb in range(NCH):
                nc.tensor.matmul(t1_ps, K_sb[jb][:, csl], lt(jb),
                                 start=(jb == 0), stop=(jb == NCH - 1))
            t = workp.tile([128, C], f16, tag=f"T1{cb}")
            if cb == 0:
                nc.scalar.copy(t, t1_ps)
            else:
                nc.vector.tensor_copy(t, t1_ps)
            T1_sb.append(t)
        ET_sb = []
        for ob in range(NCH):
            et_ps = psacc.tile([128, C], f32, tag="acc")
            osl = slice(ob * 128, (ob + 1) * 128)
            for cb in range(NCH):
                nc.tensor.matmul(et_ps, rcw(cb, osl), T1_sb[cb],
                                 start=(cb == 0), stop=False)
            nc.tensor.matmul(et_ps, aux[:, osl], a1row, start=False, stop=False)
            nc.tensor.matmul(et_ps, b2row[:, osl], aux[:, C:2 * C],
                             start=False, stop=False)
            # += I_block via matmul (keeps DVE off the critical chain)
            nc.tensor.matmul(et_ps, ident[:, 0, 0:128], ident[:, ob, :],
                             start=False, stop=True)
            t = workp.tile([128, C], f16, tag=f"ET{ob}")
            if ob == 0:
                nc.scalar.copy(t, et_ps)
            else:
                nc.vector.tensor_copy(t, et_ps)
            ET_sb.append(t)

        # phase D: d = L'(K/N)ptb + c1 a1 + c2n wgb + W_b
        col_ps = pscol.tile([128, 12], f32, tag="cols")
        for cb in range(NCH):
            csl = slice(cb * 128, (cb + 1) * 128)
            for jb in range(NCH):
                nc.tensor.matmul(col_ps[:, cb:cb + 1], K_sb[jb][:, csl],
                                 ptbc(jb),
                                 start=(jb == 0), stop=(jb == NCH - 1))
        # c2n = (sxj.ptb)/N via the sxj/N column of K_sb (1-partition out)
        for jb in range(NCH):
            nc.tensor.matmul(col_ps[0:1, 6:7], sxn(jb), ptbc(jb),
                             start=(jb == 0), stop=(jb == NCH - 1))
        kpc = rowsp.tile([128, NCH], f16, tag="kpc")
        nc.scalar.copy(kpc, col_ps[:, 0:NCH])
        c2cell = rowsp.tile([1, 1], f16, tag="c2cell")
        nc.vector.tensor_copy(c2cell, col_ps[0:1, 6:7])
        for ob in range(NCH):
            osl = slice(ob * 128, (ob + 1) * 128)
            for cb in range(NCH):
                nc.tensor.matmul(col_ps[:, 2 + ob:3 + ob], lt(cb, osl),
                                 kpc[:, cb:cb + 1],
                                 start=(cb == 0), stop=False)
            nc.tensor.matmul(col_ps[:, 2 + ob:3 + ob], a1row[:, osl],
                             aux[:, 2 * C:2 * C + 1], start=False, stop=False)
            nc.tensor.matmul(col_ps[:, 2 + ob:3 + ob],
                             aux[:, C + ob * 128:C + (ob + 1) * 128],
                             c2cell, start=False, stop=True)
        dcol = rowsp.tile([128, NCH], f32, tag="dcol")
        nc.vector.tensor_add(dcol, col_ps[:, 2:2 + NCH], sm[:, :, 2])
        hp_ctx.__exit__(None, None, None)

        # ---- phase S: analytic BN stats ------------------------------
        K2_ps = [psg8.tile([128, C + 8], f32, tag="g8", name=f"K2_ps{m}")
                 for m in range(NCH)]
        gram(K2_ps, xiw)
        K2_sb = []
        gram_copy(K2_ps, K2_sb, "K2")

        def sxin(k):  # sxi/N column (f16)
            return K2_sb[k][:, C:C + 1]

        # QT[j, c] = (Kxi/N A^T)[j, c]; P = QT .* ET; diag = sum_j P
        P_sb = []
        for jb in range(NCH):
            qt_ps = psacc.tile([128, C], f32, tag="acc")
            jsl = slice(jb * 128, (jb + 1) * 128)
            for kb in range(NCH):
                nc.tensor.matmul(qt_ps, K2_sb[kb][:, jsl], ET_sb[kb],
                                 start=(kb == 0), stop=(kb == NCH - 1))
            p = workp.tile([128, C], f16, tag=f"P{jb}")
            nc.vector.tensor_mul(p, qt_ps, ET_sb[jb])
            P_sb.append(p)
        # col_ps cols 8-9: diag(A Kxi/N A^T); cols 10-11: asx = A sxi/N
        for cb in range(NCH):
            csl = slice(cb * 128, (cb + 1) * 128)
            for jb in range(NCH):
                nc.tensor.matmul(col_ps[:, 8 + cb:9 + cb], P_sb[jb][:, csl],
                                 ones1, start=(jb == 0), stop=(jb == NCH - 1))
            for kb in range(NCH):
                nc.tensor.matmul(col_ps[:, 10 + cb:11 + cb],
                                 ET_sb[kb][:, csl], sxin(kb),
                                 start=(kb == 0), stop=(kb == NCH - 1))
        xsb = rowsp.tile([128, 4], f32, tag="xsb")
        nc.vector.tensor_copy(xsb, col_ps[:, 8:12])

        # spack = [S1n_0 | S1n_1 | S2n_0 | S2n_1]  (per-core mean/meansq)
        spack = rowsp.tile([128, 4], f32, tag="spack")
        tmp2 = rowsp.tile([128, NCH], f32, tag="tmp2")
        for cb in range(NCH):
            dc = dcol[:, cb:cb + 1]
            asxc = xsb[:, 2 + cb:3 + cb]
            nc.vector.tensor_scalar(
                out=spack[:, cb:cb + 1], in0=asxc, scalar1=dc, scalar2=0.0,
                op0=mybir.AluOpType.add, op1=mybir.AluOpType.add)
            nc.vector.tensor_scalar(
                out=tmp2[:, cb:cb + 1], in0=asxc, scalar1=2.0, scalar2=dc,
                op0=mybir.AluOpType.mult, op1=mybir.AluOpType.add)
            nc.vector.tensor_scalar(
                out=spack[:, 2 + cb:3 + cb], in0=tmp2[:, cb:cb + 1],
                scalar1=dc, scalar2=xsb[:, cb:cb + 1],
                op0=mybir.AluOpType.mult, op1=mybir.AluOpType.add)

        # ---- ONE ReduceScatter: input = own stats tiled 8x, so every --
        # ---- core's scattered block is already the full global sum ----
        cc_in = dramp.tile([NCORES * 128, 4], f32, tag="cc_in", name="cc_in")
        cc_out = dramp.tile([128, 4], f32, tag="cc_out", name="cc_out")
        nc.sync.dma_start(
            out=cc_in.rearrange("(r p) f -> p r f", p=128),
            in_=spack.unsqueeze(1).broadcast_to([128, NCORES, 4]))
        if skip_cc:
            nc.sync.dma_start(out=cc_out, in_=cc_in[0:128, :])
        else:
            nc.gpsimd.collective_compute(
                "ReduceScatter",
                mybir.AluOpType.add,
                replica_groups=[list(range(NCORES))],
                ins=[cc_in.opt()],
                outs=[cc_out.opt()],
            )
        ssum = rowsp.tile([128, 4], f32, tag="ssum")
        nc.sync.dma_start(out=ssum, in_=cc_out)

        # ---- phase Z: z = A xi (+d via ACT) -> z_t (f16) --------------
        z_t = datap.tile([128, NCH, N], f16, tag="z")
        NZT = 8
        for tp in range(NZT):
            tsl = slice(tp * 512, (tp + 1) * 512)
            for j in range(NCH):
                jsl = slice(j * 128, (j + 1) * 128)
                z_ps = psz.tile([128, 512], f32, tag="zt")
                for k in range(NCH):
                    nc.tensor.matmul(
                        z_ps, ET_sb[k][:, jsl], xi_t[:, k, tsl],
                        start=(k == 0), stop=(k == NCH - 1))
                nc.scalar.activation(
                    out=z_t[:, j, tsl], in_=z_ps,
                    func=mybir.ActivationFunctionType.Identity,
                    bias=dcol[:, j:j + 1], scale=1.0)

        # ---- affine constants: a = gamma*rsqrt(var+eps); b = mean*a-beta
        # ssum = (8*mean | 8*meansq) per chunk; fold the /8 into the ops.
        t2 = rowsp.tile([128, NCH], f32, tag="t2")
        nc.vector.tensor_mul(t2, ssum[:, 0:2], ssum[:, 0:2])
        v8 = rowsp.tile([128, NCH], f32, tag="v8")
        nc.vector.scalar_tensor_tensor(
            out=v8, in0=t2, scalar=-1.0 / NCORES, in1=ssum[:, 2:4],
            op0=mybir.AluOpType.mult, op1=mybir.AluOpType.add)
        scol = rowsp.tile([128, NCH], f32, tag="scol")
        nc.scalar.activation(
            out=scol, in_=v8, func=mybir.ActivationFunctionType.Sqrt,
            bias=eps, scale=1.0 / NCORES)
        acols = rowsp.tile([128, NCH], f32, tag="acols")
        nc.vector.reciprocal(out=acols, in_=scol)
        nc.vector.tensor_mul(acols, acols, sm[:, :, 0])
        bcols = rowsp.tile([128, NCH], f32, tag="bcols")
        nc.vector.scalar_tensor_tensor(
            out=bcols, in0=ssum[:, 0:2], scalar=1.0 / NCORES, in1=acols,
            op0=mybir.AluOpType.mult, op1=mybir.AluOpType.mult)
        nc.vector.scalar_tensor_tensor(
            out=bcols, in0=bcols, scalar=1.0, in1=sm[:, :, 1],
            op0=mybir.AluOpType.mult, op1=mybir.AluOpType.subtract)

        # ---- apply out = a*z - b on DVE (f16 4x mode), staircased -----
        PIECES = [(0, 0, 512), (1, 0, 512), (0, 512, 1536),
                  (1, 512, 1536), (0, 2048, 2048), (1, 2048, 2048)]
        for idx, (j, off, width) in enumerate(PIECES):
            qsl = slice(off, off + width)
            o16 = outp.tile([128, width], f16, tag=f"o16_{idx}")
            nc.vector.tensor_scalar(
                out=o16, in0=z_t[:, j, qsl],
                scalar1=acols[:, j:j + 1], scalar2=bcols[:, j:j + 1],
                op0=mybir.AluOpType.mult, op1=mybir.AluOpType.subtract)
            eng = nc.sync if idx % 2 == 0 else nc.scalar
            eng.dma_start(out=out_d[j * 128:(j + 1) * 128, qsl], in_=o16)


_NC_CACHE: dict = {}


def _get_nc():
    if "nc" not in _NC_CACHE:
        nc = bacc.Bacc(
            "TRN2",
            target_bir_lowering=False,
            debug=False,
            enable_asserts=True,
            num_devices=NCORES,
        )
        build_kernel(nc)
        nc.compile()
        _NC_CACHE["nc"] = nc
    return _NC_CACHE["nc"]


def _make_in_maps(inputs: dict) -> list[dict]:
    xi = np.asarray(inputs["xi"], np.float32).reshape(B, C, N)
    xj = np.asarray(inputs["xj"], np.float32).reshape(B, C, N)
    g_w = np.asarray(inputs["g_w"], np.float32)
    g_b = np.asarray(inputs["g_b"], np.float32)
    t_w = np.asarray(inputs["theta_w"], np.float32)
    t_b = np.asarray(inputs["theta_b"], np.float32)
    p_w = np.asarray(inputs["phi_w"], np.float32)
    p_b = np.asarray(inputs["phi_b"], np.float32)
    W_w = np.asarray(inputs["W_w"], np.float32)
    W_b = np.asarray(inputs["W_b"], np.float32)
    gam = np.asarray(inputs["bn_gamma"], np.float32)
    bet = np.asarray(inputs["bn_beta"], np.float32)

    def chunked(a):  # [256, F] -> [128, 2, F]
        return np.ascontiguousarray(a.reshape(2, 128, -1).transpose(1, 0, 2))

    # host-folded weight products (constant folding, fp32)
    Lp = W_w @ g_w                      # L' = W G   (device uses K/N)
    R = p_w.T @ t_w                     # R = P^T T
    wgb = W_w @ g_b
    b1 = t_w.T @ p_b
    ptb = p_w.T @ t_b
    c1 = float(p_b @ t_b)

    wm = np.zeros((128, NCH, WM_F), np.float16)
    wm[:, :, 0:C] = chunked(Lp.T)
    wm[:, :, C:2 * C] = chunked(R)
    wm[:, :, 2 * C] = ptb.reshape(2, 128).T
    aux = np.zeros((1, 2 * C + 8), np.float16)
    aux[0, 0:C] = b1.astype(np.float16)
    aux[0, C:2 * C] = wgb.astype(np.float16)
    aux[0, 2 * C] = np.float16(c1)
    sm = np.zeros((128, NCH, 3), np.float32)
    sm[:, :, 0] = gam.reshape(2, 128).T
    sm[:, :, 1] = bet.reshape(2, 128).T
    sm[:, :, 2] = W_b.reshape(2, 128).T

    def sw8(X):  # [C, N] -> fp8 SwInterleave weights [128, 16, 2, 129, 2]
        # w8[p, t, m, j, i] = X[m*128 + 127 - j, t*256 + i*128 + p]
        # (j < 128); w8[p, t, m, 128, i] = 1.0 (ones rhs column)
        Xr = X.reshape(2, 128, NT2, 2, 128)      # [m, c, t, i, p]
        w = Xr[:, ::-1].transpose(4, 2, 0, 1, 3)  # [p, t, m, j, i]
        o = np.ones((128, NT2, 2, 129, 2), np.float32)
        o[:, :, :, 0:128, :] = w
        return o.astype(ml_dtypes.float8_e4m3)

    in_maps = []
    for b in range(B):
        xib = chunked(xi[b]).astype(np.float16)      # [128,2,4096]
        in_maps.append({
            "xjw8": sw8(xj[b]), "xiw8": sw8(xi[b]), "xi": xib, "wm": wm,
            "aux": aux, "sm": sm,
        })
    return in_maps


def kernel(**inputs) -> np.ndarray:
    nc = _get_nc()
    in_maps = _make_in_maps(inputs)
    last_err = None
    for attempt in range(3):
        try:
            res = bass_utils.run_bass_kernel_spmd(
                nc, in_maps, core_ids=list(range(NCORES)),
            )
            break
        except Exception as e:  # transient device wedge: back off and retry
            last_err = e
            import time as _time
            _time.sleep(4.0 * (attempt + 1))
            try:
                import jax
                import jax.extend.backend as _jeb
                jax.clear_caches()
                _jeb.clear_backends()
            except Exception:
                pass
    else:
        raise last_err
    out = np.stack([res.results[c]["out"] for c in range(NCORES)])
    return np.ascontiguousarray(out.reshape(B, C, 64, 64).astype(np.float32))


if __name__ == "__main__":
    rng = np.random.default_rng(0)
    fake = {
        "xi": rng.standard_normal((B, C, 64, 64)).astype(np.float32),
        "xj": rng.standard_normal((B, C, 64, 64)).astype(np.float32),
        "g_w": (rng.standard_normal((C, C)) / 16).astype(np.float32),
        "g_b": (rng.standard_normal((C,)) / 16).astype(np.float32),
        "theta_w": (rng.standard_normal((C, C)) / 16).astype(np.float32),
        "theta_b": (rng.standard_normal((C,)) / 16).astype(np.float32),
        "phi_w": (rng.standard_normal((C, C)) / 16).astype(np.float32),
        "phi_b": (rng.standard_normal((C,)) / 16).astype(np.float32),
        "W_w": (rng.standard_normal((C, C)) / 16).astype(np.float32),
        "W_b": (rng.standard_normal((C,)) / 16).astype(np.float32),
        "bn_gamma": np.ones((C,), np.float32),
        "bn_beta": np.zeros((C,), np.float32),
    }
    out = kernel(**fake)
    print("out", out.shape, out.dtype, float(np.abs(out).mean()))


# revision 25
# speedup vs baseline: 1.0350x; 1.0350x over previous
"""Trainium2 Bass kernel for nn_DilatedContextAttentionModule (B=8, C=256, 64x64).

Reference, per batch element (N = 64*64 = 4096):
    g   = G xj + g_b 1^T;  th = T xi + t_b 1^T;  phi = P xj + p_b 1^T
    f   = th^T phi / N                      (N x N, linear -- NO softmax)
    y[c,n] = sum_m f[n,m] g[c,m]
    z   = W y + W_b 1^T + xi
    out = BatchNorm2d(z)                    (training-mode batch stats)

Algebraic collapse (Gram-matrix form; exact because f is linear):
    z = A xi + d 1^T,  A = I + E'
    E' = L'(K/N) R + a1 b1^T + wgb b2n^T,  K = xj xj^T  (C x C Gram)
    with host-folded L' = W G, R = P^T T, wgb = W g_b, b1 = T^T p_b,
    ptb = P^T t_b, c1 = p_b.t_b, and runtime vectors from sxj = xj @ 1:
    a1 = L'sxj/N + wgb,  b2n = R^T sxj/N,
    d = L'(K/N)ptb + c1 a1 + (sxj.ptb/N) wgb + W_b.

KEY STRUCTURE (vs the first 57.5us version):
 1. BN statistics computed ANALYTICALLY from a second Gram Kxi = xi xi^T:
        S1/N = A sxi/N + d
        S2/N = diag(A (Kxi/N) A^T) + d*(2 A sxi/N + d)
    so the cross-core ReduceScatter (cost-model floor ~15us) launches
    right after the two C x C Grams + cheap C x C algebra and overlaps
    the entire z-phase, instead of serializing after it.
 2. Both Grams run in fp8e4 with perf_mode=DoubleRowSwInterleave (the
    only DR mode walrus codegen accepts): 0.5 cycles/row with 256-deep
    contraction -> 4.2k PE cycles per Gram instead of 16.5k (f16).
    The fp8 tensor is shipped ONCE in the hardware's interleaved
    weights layout  w8[p,t,m,j,i] = X[m*128+127-j, t*256+i*128+p];
    the matmul rhs reads the SAME tile through a permuted AP view
    (p m j i -> p i m j), which yields Gram columns reversed within
    each 128-chunk; the PSUM->SBUF copy un-reverses with a stride -1
    AP at no cost.  Row sums (sxj, sxi) come from a separate fp8 ones
    rhs accumulated into a spare PSUM column.
    fp8 Gram noise measured end-to-end: rel err ~6.4e-3 (harness gate
    2e-2).
 3. Identity block of A is generated on device (affine_select), not
    shipped; weights DMA is 0.26 MB.
 4. Tail: RS -> ssum -> affine consts (Sqrt on ACT, rest DVE) ->
    out = a*z - b on DVE (f16 4x mode) in a small-first staircase with
    stores alternating SP/ACT queues.
"""

import numpy as np
import ml_dtypes

import concourse.bass as bass
import concourse.bacc as bacc
import concourse.tile as tile
from concourse import mybir
from concourse import bass_utils

B = 8
C = 256
N = 4096          # 64 * 64
NCORES = 8
NCH = 2           # channel chunks of 128
NT2 = 16          # n chunks of 256 (fp8 DoubleRow Gram)
F32 = mybir.dt.float32
F16 = mybir.dt.float16
F8 = mybir.dt.float8e4
BN_EPS = 1e-5

# wmat layout (f16, [128, 2, 516]): per channel-chunk k:
#   [0:256] L'^T rows | [256:512] R rows | [512] ptb | pad
WM_LT = slice(0, 256)
WM_RC = slice(256, 512)
WM_F = 516


def build_kernel(nc, skip_cc: bool = False) -> None:
    f32, f16 = F32, F16
    xjw_d = nc.dram_tensor("xjw8", [128, NT2, 2, 129, 2], F8,
                           kind="ExternalInput").ap()
    xiw_d = nc.dram_tensor("xiw8", [128, NT2, 2, 129, 2], F8,
                           kind="ExternalInput").ap()
    xi_d = nc.dram_tensor("xi", [128, NCH, N], f16, kind="ExternalInput").ap()
    wm_d = nc.dram_tensor("wm", [128, NCH, WM_F], f16, kind="ExternalInput").ap()
    # aux row: [b1 (256) | wgb (256) | c1 (1) | pad]
    aux_d = nc.dram_tensor("aux", [1, 2 * C + 8], f16, kind="ExternalInput").ap()
    # f32 smalls: [gamma | beta | W_b] columns  -> [128, 2, 3]
    sm_d = nc.dram_tensor("sm", [128, NCH, 3], f32, kind="ExternalInput").ap()
    out_d = nc.dram_tensor("out", [C, N], f16, kind="ExternalOutput").ap()

    with tile.TileContext(nc) as tc:
        _body(tc, xjw_d, xiw_d, xi_d, wm_d, aux_d, sm_d, out_d, skip_cc=skip_cc)


def _body(tc, xjw_d, xiw_d, xi_d, wm_d, aux_d, sm_d, out_d,
          skip_cc: bool = False):
    nc = tc.nc
    f32, f16 = F32, F16
    import contextlib
    DR = mybir.MatmulPerfMode.DoubleRowSwInterleave

    with contextlib.ExitStack() as ctx:
        constp = ctx.enter_context(tc.tile_pool(name="const", bufs=1))
        datap = ctx.enter_context(tc.tile_pool(name="data", bufs=1))
        workp = ctx.enter_context(tc.tile_pool(name="work", bufs=4))
        rowsp = ctx.enter_context(tc.tile_pool(name="rows", bufs=2))
        outp = ctx.enter_context(tc.tile_pool(name="out", bufs=1))
        # PSUM budget (8 banks): psacc 2 + psg8 2 + psz 2 + psrow 1 + pscol 1
        psacc = ctx.enter_context(tc.tile_pool(name="ps_acc", bufs=2, space="PSUM"))
        psg8 = ctx.enter_context(tc.tile_pool(name="ps_g8", bufs=2, space="PSUM"))
        psz = ctx.enter_context(tc.tile_pool(name="ps_z", bufs=2, space="PSUM"))
        psrow = ctx.enter_context(tc.tile_pool(name="ps_row", bufs=1, space="PSUM"))
        pscol = ctx.enter_context(tc.tile_pool(name="ps_col", bufs=1, space="PSUM"))
        dramp = ctx.enter_context(tc.tile_pool(name="dram", bufs=2, space="DRAM"))

        # ---- loads, all on the SP queue so the single DMA device ------
        # ---- services them in exactly this priority order -------------
        xjw = datap.tile([128, NT2, 2, 129, 2], F8, tag="xjw")
        NXJ = 2
        for h in range(NXJ):
            sl = slice(h * (NT2 // NXJ), (h + 1) * (NT2 // NXJ))
            nc.sync.dma_start(out=xjw[:, sl], in_=xjw_d[:, sl])
        wm = constp.tile([128, NCH, WM_F], f16, tag="wm")
        nc.sync.dma_start(out=wm, in_=wm_d)
        aux = constp.tile([1, 2 * C + 8], f16, tag="aux")
        nc.sync.dma_start(out=aux, in_=aux_d)
        sm = constp.tile([128, NCH, 3], f32, tag="sm")
        nc.sync.dma_start(out=sm, in_=sm_d)
        xiw = datap.tile([128, NT2, 2, 129, 2], F8, tag="xiw")
        NXI8 = 4
        for h in range(NXI8):
            sl = slice(h * (NT2 // NXI8), (h + 1) * (NT2 // NXI8))
            nc.sync.dma_start(out=xiw[:, sl], in_=xiw_d[:, sl])
        xi_t = datap.tile([128, NCH, N], f16, tag="xi")
        NXI = 4
        for h in range(NXI):
            sl = slice(h * (N // NXI), (h + 1) * (N // NXI))
            nc.sync.dma_start(out=xi_t[:, :, sl], in_=xi_d[:, :, sl])

        eps = constp.tile([128, 1], f32, tag="eps")
        nc.vector.memset(eps, BN_EPS)
        ones1 = constp.tile([128, 1], f16, tag="ones1")
        nc.vector.memset(ones1, 1.0)
        # identity rows, generated on device: ident[:, m, c] = 1 iff
        # c == m*128 + p
        ones256 = constp.tile([128, C], f16, tag="ones256")
        nc.vector.memset(ones256, 1.0)
        ident = constp.tile([128, NCH, C], f16, tag="ident")
        for m in range(NCH):
            nc.gpsimd.affine_select(
                out=ident[:, m, :], in_=ones256, pattern=[[1, C]],
                compare_op=mybir.AluOpType.is_equal, fill=0.0,
                base=-m * 128, channel_multiplier=-1)

        def lt(k, csl=slice(0, C)):
            return wm[:, k, WM_LT][:, csl]

        def rcw(k, csl=slice(0, C)):
            return wm[:, k, WM_RC][:, csl]

        def ptbc(k):
            return wm[:, k, 512:513]

        # ---- PE warm-up: hold the p-state at full clock until the ----
        # ---- first Gram chunk arrives (cold PE runs at 0.65 GHz)  ----
        warm = constp.tile([128, 640], f16, tag="warm")
        nc.gpsimd.memset(warm, 0.0)
        wps = psz.tile([128, 512], f32, tag="zt", name="warm_ps")
        nc.tensor.matmul(wps[:, 0:128], warm[:, 0:128], warm[:, 128:256],
                         start=True, stop=True)
        NWARM = 10
        for w in range(NWARM):
            nc.tensor.matmul(wps, warm[:, 0:128], warm[:, 128:640],
                             start=True, stop=True)

        def gram(ps_tiles, src):
            """fp8 DoubleRowSwInterleave Gram: src is the interleaved
            weights tile [128, NT2, 2, 129, 2] whose j=128 slot holds
            ones, so each matmul's 258-wide output carries the row sums
            at block-relative column 128 inside the SAME accumulation
            group (PSUM zero regions allow one group per bank)."""
            for t in range(NT2):
                rhs = src[:, t].rearrange("p m j i -> p i m j")
                for m in range(NCH):
                    nc.tensor.matmul(
                        ps_tiles[m][:, 0:258], src[:, t, m, 0:128, :], rhs,
                        start=(t == 0), stop=(t == NT2 - 1), perf_mode=DR)

        def gram_copy(ps_tiles, out_tiles, tagp):
            """PSUM -> SBUF f16 with 1/N scale; un-reverses the
            within-chunk column order (psum col = mb*129 + 127 - c,
            row sums at mb*129 + 128); row-sum column copied FIRST so
            consumers of sxn can start early."""
            for m in range(NCH):
                t = workp.tile([128, C + 1], f16, tag=f"{tagp}{m}")
                blk = ps_tiles[m][:, 0:258].rearrange(
                    "p (mb c) -> p mb c", mb=NCH)
                rev = blk[:, :, 0:128][:, :, ::-1]
                if m == 0:
                    nc.scalar.activation(
                        out=t[:, C:C + 1], in_=blk[:, 0, 128:129],
                        func=mybir.ActivationFunctionType.Identity,
                        scale=1.0 / N)
                    nc.scalar.activation(
                        out=t[:, 0:C], in_=rev,
                        func=mybir.ActivationFunctionType.Identity,
                        scale=1.0 / N)
                else:
                    nc.vector.tensor_scalar_mul(
                        t[:, C:C + 1], blk[:, 0, 128:129], 1.0 / N)
                    nc.vector.tensor_scalar_mul(t[:, 0:C], rev, 1.0 / N)
                out_tiles.append(t)

        # ---- phase A: K_aug = [xj|1]^T [xj|1] -> K and sxj ------------
        K_ps = [psacc.tile([128, C + 8], f32, tag="acc", name=f"K_ps{m}")
                for m in range(NCH)]
        gram(K_ps, xjw)
        K_sb = []
        gram_copy(K_ps, K_sb, "K")

        def sxn(k):  # sxj/N column (f16)
            return K_sb[k][:, C:C + 1]

        # ---- phases B/C/D: C x C algebra — high priority so the ------
        # ---- scheduler orders these PE ops ahead of the Kxi Gram -----
        # ---- matmuls (whose xiw DMA waits would head-of-line block ---
        # ---- the in-order PE sequencer) ------------------------------
        hp_ctx = tc.high_priority()
        hp_ctx.__enter__()

        # phase B: runtime rows a1 = L'sxj/N + wgb, b2n = R^T sxj/N
        rows_ps = psrow.tile([1, 2 * C], f32, tag="rows")
        for k in range(NCH):
            nc.tensor.matmul(rows_ps[:, 0:C], sxn(k), lt(k),
                             start=(k == 0), stop=(k == NCH - 1))
        for k in range(NCH):
            nc.tensor.matmul(rows_ps[:, C:2 * C], sxn(k), rcw(k),
                             start=(k == 0), stop=(k == NCH - 1))
        a1row = rowsp.tile([1, C], f16, tag="a1row")
        nc.vector.tensor_add(a1row, rows_ps[:, 0:C], aux[:, C:2 * C])
        b2row = rowsp.tile([1, C], f16, tag="b2row")
        nc.vector.tensor_copy(b2row, rows_ps[:, C:2 * C])

        # phase C: T1 = (K/N) L'^T;  ET = R^T T1 + rank1 + I
        T1_sb = []
        for cb in range(NCH):
            t1_ps = psacc.tile([128, C], f32, tag="acc")
            csl = slice(cb * 128, (cb + 1) * 128)
            for jb in range(NCH):
                nc.tensor.matmul(t1_ps, K_sb[jb][:, csl], lt(jb),
                                 start=(jb == 0), stop=(jb == NCH - 1))
            t = workp.tile([128, C], f16, tag=f"T1{cb}")
            if cb == 0:
                nc.scalar.copy(t, t1_ps)
            else:
                nc.vector.tensor_copy(t, t1_ps)
            T1_sb.append(t)
        ET_sb = []
        for ob in range(NCH):
            et_ps = psacc.tile([128, C], f32, tag="acc")
            osl = slice(ob * 128, (ob + 1) * 128)
            for cb in range(NCH):
                nc.tensor.matmul(et_ps, rcw(cb, osl), T1_sb[cb],
                                 start=(cb == 0), stop=False)
            nc.tensor.matmul(et_ps, aux[:, osl], a1row, start=False, stop=False)
            nc.tensor.matmul(et_ps, b2row[:, osl], aux[:, C:2 * C],
                             start=False, stop=False)
            # += I_block via matmul (keeps DVE off the critical chain)
            nc.tensor.matmul(et_ps, ident[:, 0, 0:128], ident[:, ob, :],
                             start=False, stop=True)
            t = workp.tile([128, C], f16, tag=f"ET{ob}")
            if ob == 0:
                nc.scalar.copy(t, et_ps)
            else:
                nc.vector.tensor_copy(t, et_ps)
            ET_sb.append(t)

        # phase D: d = L'(K/N)ptb + c1 a1 + c2n wgb + W_b
        col_ps = pscol.tile([128, 12], f32, tag="cols")
        for cb in range(NCH):
            csl = slice(cb * 128, (cb + 1) * 128)
            for jb in range(NCH):
                nc.tensor.matmul(col_ps[:, cb:cb + 1], K_sb[jb][:, csl],
                                 ptbc(jb),
                                 start=(jb == 0), stop=(jb == NCH - 1))
        # c2n = (sxj.ptb)/N via the sxj/N column of K_sb (1-partition out)
        for jb in range(NCH):
            nc.tensor.matmul(col_ps[0:1, 6:7], sxn(jb), ptbc(jb),
                             start=(jb == 0), stop=(jb == NCH - 1))
        kpc = rowsp.tile([128, NCH], f16, tag="kpc")
        nc.scalar.copy(kpc, col_ps[:, 0:NCH])
        c2cell = rowsp.tile([1, 1], f16, tag="c2cell")
        nc.vector.tensor_copy(c2cell, col_ps[0:1, 6:7])
        for ob in range(NCH):
            osl = slice(ob * 128, (ob + 1) * 128)
            for cb in range(NCH):
                nc.tensor.matmul(col_ps[:, 2 + ob:3 + ob], lt(cb, osl),
                                 kpc[:, cb:cb + 1],
                                 start=(cb == 0), stop=False)
            nc.tensor.matmul(col_ps[:, 2 + ob:3 + ob], a1row[:, osl],
                             aux[:, 2 * C:2 * C + 1], start=False, stop=False)
            nc.tensor.matmul(col_ps[:, 2 + ob:3 + ob],
                             aux[:, C + ob * 128:C + (ob + 1) * 128],
                             c2cell, start=False, stop=True)
        dcol = rowsp.tile([128, NCH], f32, tag="dcol")
        nc.vector.tensor_add(dcol, col_ps[:, 2:2 + NCH], sm[:, :, 2])
        hp_ctx.__exit__(None, None, None)

        # ---- phase S: analytic BN stats ------------------------------
        K2_ps = [psg8.tile([128, C + 8], f32, tag="g8", name=f"K2_ps{m}")
                 for m in range(NCH)]
        gram(K2_ps, xiw)
        K2_sb = []
        gram_copy(K2_ps, K2_sb, "K2")

        def sxin(k):  # sxi/N column (f16)
            return K2_sb[k][:, C:C + 1]

        # QT[j, c] = (Kxi/N A^T)[j, c]; P = QT .* ET; diag = sum_j P
        P_sb = []
        for jb in range(NCH):
            qt_ps = psacc.tile([128, C], f32, tag="acc")
            jsl = slice(jb * 128, (jb + 1) * 128)
            for kb in range(NCH):
                nc.tensor.matmul(qt_ps, K2_sb[kb][:, jsl], ET_sb[kb],
                                 start=(kb == 0), stop=(kb == NCH - 1))
            p = workp.tile([128, C], f16, tag=f"P{jb}")
            nc.vector.tensor_mul(p, qt_ps, ET_sb[jb])
            P_sb.append(p)
        # col_ps cols 8-9: diag(A Kxi/N A^T); cols 10-11: asx = A sxi/N
        for cb in range(NCH):
            csl = slice(cb * 128, (cb + 1) * 128)
            for jb in range(NCH):
                nc.tensor.matmul(col_ps[:, 8 + cb:9 + cb], P_sb[jb][:, csl],
                                 ones1, start=(jb == 0), stop=(jb == NCH - 1))
            for kb in range(NCH):
                nc.tensor.matmul(col_ps[:, 10 + cb:11 + cb],
                                 ET_sb[kb][:, csl], sxin(kb),
                                 start=(kb == 0), stop=(kb == NCH - 1))
        xsb = rowsp.tile([128, 4], f32, tag="xsb")
        nc.vector.tensor_copy(xsb, col_ps[:, 8:12])

        # spack = [S1n_0 | S1n_1 | S2n_0 | S2n_1]  (per-core mean/meansq)
        spack = rowsp.tile([128, 4], f32, tag="spack")
        tmp2 = rowsp.tile([128, NCH], f32, tag="tmp2")
        for cb in range(NCH):
            dc = dcol[:, cb:cb + 1]
            asxc = xsb[:, 2 + cb:3 + cb]
            nc.vector.tensor_scalar(
                out=spack[:, cb:cb + 1], in0=asxc, scalar1=dc, scalar2=0.0,
                op0=mybir.AluOpType.add, op1=mybir.AluOpType.add)
            nc.vector.tensor_scalar(
                out=tmp2[:, cb:cb + 1], in0=asxc, scalar1=2.0, scalar2=dc,
                op0=mybir.AluOpType.mult, op1=mybir.AluOpType.add)
            nc.vector.tensor_scalar(
                out=spack[:, 2 + cb:3 + cb], in0=tmp2[:, cb:cb + 1],
                scalar1=dc, scalar2=xsb[:, cb:cb + 1],
                op0=mybir.AluOpType.mult, op1=mybir.AluOpType.add)

        # ---- ONE ReduceScatter: input = own stats tiled 8x, so every --
        # ---- core's scattered block is already the full global sum ----
        cc_in = dramp.tile([NCORES * 128, 4], f32, tag="cc_in", name="cc_in")
        cc_out = dramp.tile([128, 4], f32, tag="cc_out", name="cc_out")
        nc.sync.dma_start(
            out=cc_in.rearrange("(r p) f -> p r f", p=128),
            in_=spack.unsqueeze(1).broadcast_to([128, NCORES, 4]))
        if skip_cc:
            nc.sync.dma_start(out=cc_out, in_=cc_in[0:128, :])
        else:
            nc.gpsimd.collective_compute(
                "ReduceScatter",
                mybir.AluOpType.add,
                replica_groups=[list(range(NCORES))],
                ins=[cc_in.opt()],
                outs=[cc_out.opt()],
            )
        ssum = rowsp.tile([128, 4], f32, tag="ssum")
        nc.sync.dma_start(out=ssum, in_=cc_out)

        # ---- phase Z: z = A xi (+d via ACT) -> z_t (f16) --------------
        z_t = datap.tile([128, NCH, N], f16, tag="z")
        NZT = 8
        for tp in range(NZT):
            tsl = slice(tp * 512, (tp + 1) * 512)
            for j in range(NCH):
                jsl = slice(j * 128, (j + 1) * 128)
                z_ps = psz.tile([128, 512], f32, tag="zt")
                for k in range(NCH):
                    nc.tensor.matmul(
                        z_ps, ET_sb[k][:, jsl], xi_t[:, k, tsl],
                        start=(k == 0), stop=(k == NCH - 1))
                nc.scalar.activation(
                    out=z_t[:, j, tsl], in_=z_ps,
                    func=mybir.ActivationFunctionType.Identity,
                    bias=dcol[:, j:j + 1], scale=1.0)

        # ---- affine constants: a = gamma*rsqrt(var+eps); b = mean*a-beta
        # ssum = (8*mean | 8*meansq) per chunk; fold the /8 into the ops.
        t2 = rowsp.tile([128, NCH], f32, tag="t2")
        nc.vector.tensor_mul(t2, ssum[:, 0:2], ssum[:, 0:2])
        v8 = rowsp.tile([128, NCH], f32, tag="v8")
        nc.vector.scalar_tensor_tensor(
            out=v8, in0=t2, scalar=-1.0 / NCORES, in1=ssum[:, 2:4],
            op0=mybir.AluOpType.mult, op1=mybir.AluOpType.add)
        scol = rowsp.tile([128, NCH], f32, tag="scol")
        nc.scalar.activation(
            out=scol, in_=v8, func=mybir.ActivationFunctionType.Sqrt,
            bias=eps, scale=1.0 / NCORES)
        acols = rowsp.tile([128, NCH], f32, tag="acols")
        nc.vector.reciprocal(out=acols, in_=scol)
        nc.vector.tensor_mul(acols, acols, sm[:, :, 0])
        bcols = rowsp.tile([128, NCH], f32, tag="bcols")
        nc.vector.scalar_tensor_tensor(
            out=bcols, in0=ssum[:, 0:2], scalar=1.0 / NCORES, in1=acols,
            op0=mybir.AluOpType.mult, op1=mybir.AluOpType.mult)
        nc.vector.scalar_tensor_tensor(
            out=bcols, in0=bcols, scalar=1.0, in1=sm[:, :, 1],
            op0=mybir.AluOpType.mult, op1=mybir.AluOpType.subtract)

        # ---- apply out = a*z - b on DVE (f16 4x mode), staircased -----
        PIECES = [(0, 0, 512), (1, 0, 512), (0, 512, 1536),
                  (1, 512, 1536), (0, 2048, 2048), (1, 2048, 2048)]
        for idx, (j, off, width) in enumerate(PIECES):
            qsl = slice(off, off + width)
            o16 = outp.tile([128, width], f16, tag=f"o16_{idx}")
            nc.vector.tensor_scalar(
                out=o16, in0=z_t[:, j, qsl],
                scalar1=acols[:, j:j + 1], scalar2=bcols[:, j:j + 1],
                op0=mybir.AluOpType.mult, op1=mybir.AluOpType.subtract)
            eng = nc.sync if idx % 2 == 0 else nc.scalar
            eng.dma_start(out=out_d[j * 128:(j + 1) * 128, qsl], in_=o16)


_NC_CACHE: dict = {}


def _get_nc():
    if "nc" not in _NC_CACHE:
        nc = bacc.Bacc(
            "TRN2",
            target_bir_lowering=False,
            debug=False,
            enable_asserts=True,
            num_devices=NCORES,
        )
        build_kernel(nc)
        nc.compile()
        _NC_CACHE["nc"] = nc
    return _NC_CACHE["nc"]


def _make_in_maps(inputs: dict) -> list[dict]:
    xi = np.asarray(inputs["xi"], np.float32).reshape(B, C, N)
    xj = np.asarray(inputs["xj"], np.float32).reshape(B, C, N)
    g_w = np.asarray(inputs["g_w"], np.float32)
    g_b = np.asarray(inputs["g_b"], np.float32)
    t_w = np.asarray(inputs["theta_w"], np.float32)
    t_b = np.asarray(inputs["theta_b"], np.float32)
    p_w = np.asarray(inputs["phi_w"], np.float32)
    p_b = np.asarray(inputs["phi_b"], np.float32)
    W_w = np.asarray(inputs["W_w"], np.float32)
    W_b = np.asarray(inputs["W_b"], np.float32)
    gam = np.asarray(inputs["bn_gamma"], np.float32)
    bet = np.asarray(inputs["bn_beta"], np.float32)

    def chunked(a):  # [256, F] -> [128, 2, F]
        return np.ascontiguousarray(a.reshape(2, 128, -1).transpose(1, 0, 2))

    # host-folded weight products (constant folding, fp32)
    Lp = W_w @ g_w                      # L' = W G   (device uses K/N)
    R = p_w.T @ t_w                     # R = P^T T
    wgb = W_w @ g_b
    b1 = t_w.T @ p_b
    ptb = p_w.T @ t_b
    c1 = float(p_b @ t_b)

    wm = np.zeros((128, NCH, WM_F), np.float16)
    wm[:, :, 0:C] = chunked(Lp.T)
    wm[:, :, C:2 * C] = chunked(R)
    wm[:, :, 2 * C] = ptb.reshape(2, 128).T
    aux = np.zeros((1, 2 * C + 8), np.float16)
    aux[0, 0:C] = b1.astype(np.float16)
    aux[0, C:2 * C] = wgb.astype(np.float16)
    aux[0, 2 * C] = np.float16(c1)
    sm = np.zeros((128, NCH, 3), np.float32)
    sm[:, :, 0] = gam.reshape(2, 128).T
    sm[:, :, 1] = bet.reshape(2, 128).T
    sm[:, :, 2] = W_b.reshape(2, 128).T

    def sw8(X):  # [C, N] -> fp8 SwInterleave weights [128, 16, 2, 129, 2]
        # w8[p, t, m, j, i] = X[m*128 + 127 - j, t*256 + i*128 + p]
        # (j < 128); w8[p, t, m, 128, i] = 1.0 (ones rhs column)
        Xr = X.reshape(2, 128, NT2, 2, 128)      # [m, c, t, i, p]
        w = Xr[:, ::-1].transpose(4, 2, 0, 1, 3)  # [p, t, m, j, i]
        o = np.ones((128, NT2, 2, 129, 2), np.float32)
        o[:, :, :, 0:128, :] = w
        return o.astype(ml_dtypes.float8_e4m3)

    in_maps = []
    for b in range(B):
        xib = chunked(xi[b]).astype(np.float16)      # [128,2,4096]
        in_maps.append({
            "xjw8": sw8(xj[b]), "xiw8": sw8(xi[b]), "xi": xib, "wm": wm,
            "aux": aux, "sm": sm,
        })
    return in_maps


def kernel(**inputs) -> np.ndarray:
    nc = _get_nc()
    in_maps = _make_in_maps(inputs)
    last_err = None
    for attempt in range(3):
        try:
            res = bass_utils.run_bass_kernel_spmd(
                nc, in_maps, core_ids=list(range(NCORES)),
            )
            break
        except Exception as e:  # transient device wedge: back off and retry
            last_err = e
            import time as _time
            _time.sleep(4.0 * (attempt + 1))
            try:
                import jax
                import jax.extend.backend as _jeb
                jax.clear_caches()
                _jeb.clear_backends()
            except Exception:
                pass
    else:
        raise last_err
    out = np.stack([res.results[c]["out"] for c in range(NCORES)])
    return np.ascontiguousarray(out.reshape(B, C, 64, 64).astype(np.float32))


if __name__ == "__main__":
    rng = np.random.default_rng(0)
    fake = {
        "xi": rng.standard_normal((B, C, 64, 64)).astype(np.float32),
        "xj": rng.standard_normal((B, C, 64, 64)).astype(np.float32),
        "g_w": (rng.standard_normal((C, C)) / 16).astype(np.float32),
        "g_b": (rng.standard_normal((C,)) / 16).astype(np.float32),
        "theta_w": (rng.standard_normal((C, C)) / 16).astype(np.float32),
        "theta_b": (rng.standard_normal((C,)) / 16).astype(np.float32),
        "phi_w": (rng.standard_normal((C, C)) / 16).astype(np.float32),
        "phi_b": (rng.standard_normal((C,)) / 16).astype(np.float32),
        "W_w": (rng.standard_normal((C, C)) / 16).astype(np.float32),
        "W_b": (rng.standard_normal((C,)) / 16).astype(np.float32),
        "bn_gamma": np.ones((C,), np.float32),
        "bn_beta": np.zeros((C,), np.float32),
    }
    out = kernel(**fake)
    print("out", out.shape, out.dtype, float(np.abs(out).mean()))


# revision 32
# speedup vs baseline: 1.0557x; 1.0200x over previous
"""Trainium2 Bass kernel for nn_DilatedContextAttentionModule (B=8, C=256, 64x64).

Reference, per batch element (N = 64*64 = 4096):
    g   = G xj + g_b 1^T;  th = T xi + t_b 1^T;  phi = P xj + p_b 1^T
    f   = th^T phi / N                      (N x N, linear -- NO softmax)
    y[c,n] = sum_m f[n,m] g[c,m]
    z   = W y + W_b 1^T + xi
    out = BatchNorm2d(z)                    (training-mode batch stats)

Algebraic collapse (Gram-matrix form; exact because f is linear):
    z = A xi + d 1^T,  A = I + E'
    E' = L'(K/N) R + a1 b1^T + wgb b2n^T,  K = xj xj^T  (C x C Gram)
    with host-folded L' = W G, R = P^T T, wgb = W g_b, b1 = T^T p_b,
    ptb = P^T t_b, c1 = p_b.t_b, and runtime vectors from sxj = xj @ 1:
    a1 = L'sxj/N + wgb,  b2n = R^T sxj/N,
    d = L'(K/N)ptb + c1 a1 + (sxj.ptb/N) wgb + W_b.

KEY STRUCTURE (vs the first 57.5us version):
 1. BN statistics computed ANALYTICALLY from a second Gram Kxi = xi xi^T:
        S1/N = A sxi/N + d
        S2/N = diag(A (Kxi/N) A^T) + d*(2 A sxi/N + d)
    so the cross-core ReduceScatter (cost-model floor ~15us) launches
    right after the two C x C Grams + cheap C x C algebra and overlaps
    the entire z-phase, instead of serializing after it.
 2. Both Grams run in fp8e4 with perf_mode=DoubleRowSwInterleave (the
    only DR mode walrus codegen accepts): 0.5 cycles/row with 256-deep
    contraction -> 4.2k PE cycles per Gram instead of 16.5k (f16).
    The fp8 tensor is shipped ONCE in the hardware's interleaved
    weights layout  w8[p,t,m,j,i] = X[m*128+127-j, t*256+i*128+p];
    the matmul rhs reads the SAME tile through a permuted AP view
    (p m j i -> p i m j), which yields Gram columns reversed within
    each 128-chunk; the PSUM->SBUF copy un-reverses with a stride -1
    AP at no cost.  Row sums (sxj, sxi) come from a separate fp8 ones
    rhs accumulated into a spare PSUM column.
    fp8 Gram noise measured end-to-end: rel err ~6.4e-3 (harness gate
    2e-2).
 3. Identity block of A is generated on device (affine_select), not
    shipped; weights DMA is 0.26 MB.
 4. Tail: RS -> ssum -> affine consts (Sqrt on ACT, rest DVE) ->
    out = a*z - b on DVE (f16 4x mode) in a small-first staircase with
    stores alternating SP/ACT queues.
"""

import numpy as np
import ml_dtypes

import concourse.bass as bass
import concourse.bacc as bacc
import concourse.tile as tile
from concourse import mybir
from concourse import bass_utils

B = 8
C = 256
N = 4096          # 64 * 64
NCORES = 8
NCH = 2           # channel chunks of 128
NT2 = 16          # n chunks of 256 (fp8 DoubleRow Gram)
F32 = mybir.dt.float32
F16 = mybir.dt.float16
F8 = mybir.dt.float8e4
BN_EPS = 1e-5

# wmat layout (f16, [128, 2, 516]): per channel-chunk k:
#   [0:256] L'^T rows | [256:512] R rows | [512] ptb | pad
WM_LT = slice(0, 256)
WM_RC = slice(256, 512)
WM_F = 516


def build_kernel(nc, skip_cc: bool = False) -> None:
    f32, f16 = F32, F16
    xjw_d = nc.dram_tensor("xjw8", [128, NT2, 2, 129, 2], F8,
                           kind="ExternalInput").ap()
    xiw_d = nc.dram_tensor("xiw8", [128, NT2, 2, 129, 2], F8,
                           kind="ExternalInput").ap()
    xi_d = nc.dram_tensor("xi", [128, NCH, N], f16, kind="ExternalInput").ap()
    wm_d = nc.dram_tensor("wm", [128, NCH, WM_F], f16, kind="ExternalInput").ap()
    # aux row: [b1 (256) | wgb (256) | c1 (1) | pad]
    aux_d = nc.dram_tensor("aux", [1, 2 * C + 8], f16, kind="ExternalInput").ap()
    # f32 smalls: [gamma | beta | W_b] columns  -> [128, 2, 3]
    sm_d = nc.dram_tensor("sm", [128, NCH, 3], f32, kind="ExternalInput").ap()
    out_d = nc.dram_tensor("out", [C, N], f16, kind="ExternalOutput").ap()

    with tile.TileContext(nc) as tc:
        _body(tc, xjw_d, xiw_d, xi_d, wm_d, aux_d, sm_d, out_d, skip_cc=skip_cc)


def _body(tc, xjw_d, xiw_d, xi_d, wm_d, aux_d, sm_d, out_d,
          skip_cc: bool = False):
    nc = tc.nc
    f32, f16 = F32, F16
    import contextlib
    DR = mybir.MatmulPerfMode.DoubleRowSwInterleave

    with contextlib.ExitStack() as ctx:
        constp = ctx.enter_context(tc.tile_pool(name="const", bufs=1))
        datap = ctx.enter_context(tc.tile_pool(name="data", bufs=1))
        workp = ctx.enter_context(tc.tile_pool(name="work", bufs=4))
        rowsp = ctx.enter_context(tc.tile_pool(name="rows", bufs=2))
        outp = ctx.enter_context(tc.tile_pool(name="out", bufs=1))
        # PSUM budget (8 banks): psacc 2 + psg8 2 + psz 2 + psrow 1 + pscol 1
        psacc = ctx.enter_context(tc.tile_pool(name="ps_acc", bufs=2, space="PSUM"))
        psg8 = ctx.enter_context(tc.tile_pool(name="ps_g8", bufs=2, space="PSUM"))
        psz = ctx.enter_context(tc.tile_pool(name="ps_z", bufs=2, space="PSUM"))
        psrow = ctx.enter_context(tc.tile_pool(name="ps_row", bufs=1, space="PSUM"))
        pscol = ctx.enter_context(tc.tile_pool(name="ps_col", bufs=1, space="PSUM"))
        dramp = ctx.enter_context(tc.tile_pool(name="dram", bufs=2, space="DRAM"))

        # ---- loads, all on the SP queue so the single DMA device ------
        # ---- services them in exactly this priority order -------------
        xjw = datap.tile([128, NT2, 2, 129, 2], F8, tag="xjw")
        NXJ = 2
        for h in range(NXJ):
            sl = slice(h * (NT2 // NXJ), (h + 1) * (NT2 // NXJ))
            nc.sync.dma_start(out=xjw[:, sl], in_=xjw_d[:, sl])
        wm = constp.tile([128, NCH, WM_F], f16, tag="wm")
        nc.sync.dma_start(out=wm, in_=wm_d)
        aux = constp.tile([1, 2 * C + 8], f16, tag="aux")
        nc.sync.dma_start(out=aux, in_=aux_d)
        sm = constp.tile([128, NCH, 3], f32, tag="sm")
        nc.sync.dma_start(out=sm, in_=sm_d)
        xiw = datap.tile([128, NT2, 2, 129, 2], F8, tag="xiw")
        NXI8 = 4
        for h in range(NXI8):
            sl = slice(h * (NT2 // NXI8), (h + 1) * (NT2 // NXI8))
            nc.sync.dma_start(out=xiw[:, sl], in_=xiw_d[:, sl])
        xi_t = datap.tile([128, NCH, N], f16, tag="xi")
        NXI = 4
        for h in range(NXI):
            sl = slice(h * (N // NXI), (h + 1) * (N // NXI))
            nc.sync.dma_start(out=xi_t[:, :, sl], in_=xi_d[:, :, sl])

        eps = constp.tile([128, 1], f32, tag="eps")
        nc.vector.memset(eps, BN_EPS)
        ones1 = constp.tile([128, 1], f16, tag="ones1")
        nc.vector.memset(ones1, 1.0)
        # identity rows, generated on device: ident[:, m, c] = 1 iff
        # c == m*128 + p
        ones256 = constp.tile([128, C], f16, tag="ones256")
        nc.vector.memset(ones256, 1.0)
        ident = constp.tile([128, NCH, C], f16, tag="ident")
        for m in range(NCH):
            nc.gpsimd.affine_select(
                out=ident[:, m, :], in_=ones256, pattern=[[1, C]],
                compare_op=mybir.AluOpType.is_equal, fill=0.0,
                base=-m * 128, channel_multiplier=-1)

        def lt(k, csl=slice(0, C)):
            return wm[:, k, WM_LT][:, csl]

        def rcw(k, csl=slice(0, C)):
            return wm[:, k, WM_RC][:, csl]

        def ptbc(k):
            return wm[:, k, 512:513]

        # ---- PE warm-up: hold the p-state at full clock until the ----
        # ---- first Gram chunk arrives (cold PE runs at 0.65 GHz)  ----
        warm = constp.tile([128, 640], f16, tag="warm")
        nc.gpsimd.memset(warm, 0.0)
        wps = psz.tile([128, 512], f32, tag="zt", name="warm_ps")
        nc.tensor.matmul(wps[:, 0:128], warm[:, 0:128], warm[:, 128:256],
                         start=True, stop=True)
        NWARM = 6
        for w in range(NWARM):
            nc.tensor.matmul(wps, warm[:, 0:128], warm[:, 128:640],
                             start=True, stop=True)

        def gram(ps_tiles, src):
            """fp8 DoubleRowSwInterleave Gram: src is the interleaved
            weights tile [128, NT2, 2, 129, 2] whose j=128 slot holds
            ones, so each matmul's 258-wide output carries the row sums
            at block-relative column 128 inside the SAME accumulation
            group (PSUM zero regions allow one group per bank)."""
            for t in range(NT2):
                rhs = src[:, t].rearrange("p m j i -> p i m j")
                for m in range(NCH):
                    nc.tensor.matmul(
                        ps_tiles[m][:, 0:258], src[:, t, m, 0:128, :], rhs,
                        start=(t == 0), stop=(t == NT2 - 1), perf_mode=DR)

        def gram_copy(ps_tiles, out_tiles, tagp):
            """PSUM -> SBUF f16 with 1/N scale; un-reverses the
            within-chunk column order (psum col = mb*129 + 127 - c,
            row sums at mb*129 + 128); row-sum column copied FIRST so
            consumers of sxn can start early."""
            for m in range(NCH):
                t = workp.tile([128, C + 1], f16, tag=f"{tagp}{m}")
                blk = ps_tiles[m][:, 0:258].rearrange(
                    "p (mb c) -> p mb c", mb=NCH)
                rev = blk[:, :, 0:128][:, :, ::-1]
                if m == 0:
                    nc.scalar.activation(
                        out=t[:, C:C + 1], in_=blk[:, 0, 128:129],
                        func=mybir.ActivationFunctionType.Identity,
                        scale=1.0 / N)
                    nc.scalar.activation(
                        out=t[:, 0:C], in_=rev,
                        func=mybir.ActivationFunctionType.Identity,
                        scale=1.0 / N)
                else:
                    nc.vector.tensor_scalar_mul(
                        t[:, C:C + 1], blk[:, 0, 128:129], 1.0 / N)
                    nc.vector.tensor_scalar_mul(t[:, 0:C], rev, 1.0 / N)
                out_tiles.append(t)

        # ---- phase A: K_aug = [xj|1]^T [xj|1] -> K and sxj ------------
        K_ps = [psacc.tile([128, C + 8], f32, tag="acc", name=f"K_ps{m}")
                for m in range(NCH)]
        gram(K_ps, xjw)
        K_sb = []
        gram_copy(K_ps, K_sb, "K")

        def sxn(k):  # sxj/N column (f16)
            return K_sb[k][:, C:C + 1]

        # ---- phases B/C/D: C x C algebra — high priority so the ------
        # ---- scheduler orders these PE ops ahead of the Kxi Gram -----
        # ---- matmuls (whose xiw DMA waits would head-of-line block ---
        # ---- the in-order PE sequencer) ------------------------------
        hp_ctx = tc.high_priority()
        hp_ctx.__enter__()

        # phase B: runtime rows a1 = L'sxj/N + wgb, b2n = R^T sxj/N
        rows_ps = psrow.tile([1, 2 * C], f32, tag="rows")
        for k in range(NCH):
            nc.tensor.matmul(rows_ps[:, 0:C], sxn(k), lt(k),
                             start=(k == 0), stop=(k == NCH - 1))
        for k in range(NCH):
            nc.tensor.matmul(rows_ps[:, C:2 * C], sxn(k), rcw(k),
                             start=(k == 0), stop=(k == NCH - 1))
        a1row = rowsp.tile([1, C], f16, tag="a1row")
        nc.vector.tensor_add(a1row, rows_ps[:, 0:C], aux[:, C:2 * C])
        b2row = rowsp.tile([1, C], f16, tag="b2row")
        nc.vector.tensor_copy(b2row, rows_ps[:, C:2 * C])

        # phase C: T1 = (K/N) L'^T;  ET = R^T T1 + rank1 + I
        T1_sb = []
        for cb in range(NCH):
            t1_ps = psacc.tile([128, C], f32, tag="acc")
            csl = slice(cb * 128, (cb + 1) * 128)
            for jb in range(NCH):
                nc.tensor.matmul(t1_ps, K_sb[jb][:, csl], lt(jb),
                                 start=(jb == 0), stop=(jb == NCH - 1))
            t = workp.tile([128, C], f16, tag=f"T1{cb}")
            if cb == 0:
                nc.scalar.copy(t, t1_ps)
            else:
                nc.vector.tensor_copy(t, t1_ps)
            T1_sb.append(t)
        ET_sb = []
        for ob in range(NCH):
            et_ps = psacc.tile([128, C], f32, tag="acc")
            osl = slice(ob * 128, (ob + 1) * 128)
            for cb in range(NCH):
                nc.tensor.matmul(et_ps, rcw(cb, osl), T1_sb[cb],
                                 start=(cb == 0), stop=False)
            nc.tensor.matmul(et_ps, aux[:, osl], a1row, start=False, stop=False)
            nc.tensor.matmul(et_ps, b2row[:, osl], aux[:, C:2 * C],
                             start=False, stop=False)
            # += I_block via matmul (keeps DVE off the critical chain)
            nc.tensor.matmul(et_ps, ident[:, 0, 0:128], ident[:, ob, :],
                             start=False, stop=True)
            t = workp.tile([128, C], f16, tag=f"ET{ob}")
            if ob == 0:
                nc.scalar.copy(t, et_ps)
            else:
                nc.vector.tensor_copy(t, et_ps)
            ET_sb.append(t)

        # phase D: d = L'(K/N)ptb + c1 a1 + c2n wgb + W_b
        col_ps = pscol.tile([128, 12], f32, tag="cols")
        for cb in range(NCH):
            csl = slice(cb * 128, (cb + 1) * 128)
            for jb in range(NCH):
                nc.tensor.matmul(col_ps[:, cb:cb + 1], K_sb[jb][:, csl],
                                 ptbc(jb),
                                 start=(jb == 0), stop=(jb == NCH - 1))
        # c2n = (sxj.ptb)/N via the sxj/N column of K_sb (1-partition out)
        for jb in range(NCH):
            nc.tensor.matmul(col_ps[0:1, 6:7], sxn(jb), ptbc(jb),
                             start=(jb == 0), stop=(jb == NCH - 1))
        kpc = rowsp.tile([128, NCH], f16, tag="kpc")
        nc.scalar.copy(kpc, col_ps[:, 0:NCH])
        c2cell = rowsp.tile([1, 1], f16, tag="c2cell")
        nc.vector.tensor_copy(c2cell, col_ps[0:1, 6:7])
        for ob in range(NCH):
            osl = slice(ob * 128, (ob + 1) * 128)
            for cb in range(NCH):
                nc.tensor.matmul(col_ps[:, 2 + ob:3 + ob], lt(cb, osl),
                                 kpc[:, cb:cb + 1],
                                 start=(cb == 0), stop=False)
            nc.tensor.matmul(col_ps[:, 2 + ob:3 + ob], a1row[:, osl],
                             aux[:, 2 * C:2 * C + 1], start=False, stop=False)
            nc.tensor.matmul(col_ps[:, 2 + ob:3 + ob],
                             aux[:, C + ob * 128:C + (ob + 1) * 128],
                             c2cell, start=False, stop=True)
        dcol = rowsp.tile([128, NCH], f32, tag="dcol")
        nc.vector.tensor_add(dcol, col_ps[:, 2:2 + NCH], sm[:, :, 2])
        hp_ctx.__exit__(None, None, None)

        # ---- phase S: analytic BN stats ------------------------------
        K2_ps = [psg8.tile([128, C + 8], f32, tag="g8", name=f"K2_ps{m}")
                 for m in range(NCH)]
        gram(K2_ps, xiw)
        K2_sb = []
        gram_copy(K2_ps, K2_sb, "K2")

        def sxin(k):  # sxi/N column (f16)
            return K2_sb[k][:, C:C + 1]

        # QT[j, c] = (Kxi/N A^T)[j, c]; P = QT .* ET; diag = sum_j P
        P_sb = []
        for jb in range(NCH):
            qt_ps = psacc.tile([128, C], f32, tag="acc")
            jsl = slice(jb * 128, (jb + 1) * 128)
            for kb in range(NCH):
                nc.tensor.matmul(qt_ps, K2_sb[kb][:, jsl], ET_sb[kb],
                                 start=(kb == 0), stop=(kb == NCH - 1))
            p = workp.tile([128, C], f16, tag=f"P{jb}")
            nc.vector.tensor_mul(p, qt_ps, ET_sb[jb])
            P_sb.append(p)
        # col_ps cols 8-9: diag(A Kxi/N A^T); cols 10-11: asx = A sxi/N
        for cb in range(NCH):
            csl = slice(cb * 128, (cb + 1) * 128)
            for jb in range(NCH):
                nc.tensor.matmul(col_ps[:, 8 + cb:9 + cb], P_sb[jb][:, csl],
                                 ones1, start=(jb == 0), stop=(jb == NCH - 1))
            for kb in range(NCH):
                nc.tensor.matmul(col_ps[:, 10 + cb:11 + cb],
                                 ET_sb[kb][:, csl], sxin(kb),
                                 start=(kb == 0), stop=(kb == NCH - 1))
        xsb = rowsp.tile([128, 4], f32, tag="xsb")
        nc.vector.tensor_copy(xsb, col_ps[:, 8:12])

        # spack = [S1n_0 | S1n_1 | S2n_0 | S2n_1]  (per-core mean/meansq)
        spack = rowsp.tile([128, 4], f32, tag="spack")
        tmp2 = rowsp.tile([128, NCH], f32, tag="tmp2")
        for cb in range(NCH):
            dc = dcol[:, cb:cb + 1]
            asxc = xsb[:, 2 + cb:3 + cb]
            nc.vector.tensor_scalar(
                out=spack[:, cb:cb + 1], in0=asxc, scalar1=dc, scalar2=0.0,
                op0=mybir.AluOpType.add, op1=mybir.AluOpType.add)
            nc.vector.tensor_scalar(
                out=tmp2[:, cb:cb + 1], in0=asxc, scalar1=2.0, scalar2=dc,
                op0=mybir.AluOpType.mult, op1=mybir.AluOpType.add)
            nc.vector.tensor_scalar(
                out=spack[:, 2 + cb:3 + cb], in0=tmp2[:, cb:cb + 1],
                scalar1=dc, scalar2=xsb[:, cb:cb + 1],
                op0=mybir.AluOpType.mult, op1=mybir.AluOpType.add)

        # ---- ONE ReduceScatter: input = own stats tiled 8x, so every --
        # ---- core's scattered block is already the full global sum ----
        cc_in = dramp.tile([NCORES * 128, 4], f32, tag="cc_in", name="cc_in")
        cc_out = dramp.tile([128, 4], f32, tag="cc_out", name="cc_out")
        nc.sync.dma_start(
            out=cc_in.rearrange("(r p) f -> p r f", p=128),
            in_=spack.unsqueeze(1).broadcast_to([128, NCORES, 4]))
        if skip_cc:
            nc.sync.dma_start(out=cc_out, in_=cc_in[0:128, :])
        else:
            nc.gpsimd.collective_compute(
                "ReduceScatter",
                mybir.AluOpType.add,
                replica_groups=[list(range(NCORES))],
                ins=[cc_in.opt()],
                outs=[cc_out.opt()],
            )
        ssum = rowsp.tile([128, 4], f32, tag="ssum")
        nc.sync.dma_start(out=ssum, in_=cc_out)

        # ---- phase Z: z = A xi (+d via ACT) -> z_t (f16) --------------
        z_t = datap.tile([128, NCH, N], f16, tag="z")
        NZT = 8
        for tp in range(NZT):
            tsl = slice(tp * 512, (tp + 1) * 512)
            for j in range(NCH):
                jsl = slice(j * 128, (j + 1) * 128)
                z_ps = psz.tile([128, 512], f32, tag="zt")
                for k in range(NCH):
                    nc.tensor.matmul(
                        z_ps, ET_sb[k][:, jsl], xi_t[:, k, tsl],
                        start=(k == 0), stop=(k == NCH - 1))
                nc.scalar.activation(
                    out=z_t[:, j, tsl], in_=z_ps,
                    func=mybir.ActivationFunctionType.Identity,
                    bias=dcol[:, j:j + 1], scale=1.0)

        # ---- affine constants: a = gamma*rsqrt(var+eps); b = mean*a-beta
        # ssum = (8*mean | 8*meansq) per chunk; fold the /8 into the ops.
        t2 = rowsp.tile([128, NCH], f32, tag="t2")
        nc.vector.tensor_mul(t2, ssum[:, 0:2], ssum[:, 0:2])
        v8 = rowsp.tile([128, NCH], f32, tag="v8")
        nc.vector.scalar_tensor_tensor(
            out=v8, in0=t2, scalar=-1.0 / NCORES, in1=ssum[:, 2:4],
            op0=mybir.AluOpType.mult, op1=mybir.AluOpType.add)
        scol = rowsp.tile([128, NCH], f32, tag="scol")
        nc.scalar.activation(
            out=scol, in_=v8, func=mybir.ActivationFunctionType.Sqrt,
            bias=eps, scale=1.0 / NCORES)
        acols = rowsp.tile([128, NCH], f32, tag="acols")
        nc.vector.reciprocal(out=acols, in_=scol)
        nc.vector.tensor_mul(acols, acols, sm[:, :, 0])
        bcols = rowsp.tile([128, NCH], f32, tag="bcols")
        nc.vector.scalar_tensor_tensor(
            out=bcols, in0=ssum[:, 0:2], scalar=1.0 / NCORES, in1=acols,
            op0=mybir.AluOpType.mult, op1=mybir.AluOpType.mult)
        nc.vector.scalar_tensor_tensor(
            out=bcols, in0=bcols, scalar=1.0, in1=sm[:, :, 1],
            op0=mybir.AluOpType.mult, op1=mybir.AluOpType.subtract)

        # ---- apply out = a*z - b on DVE (f16 4x mode), staircased -----
        PIECES = [(0, 0, 512), (1, 0, 512), (0, 512, 1536),
                  (1, 512, 1536), (0, 2048, 2048), (1, 2048, 2048)]
        for idx, (j, off, width) in enumerate(PIECES):
            qsl = slice(off, off + width)
            o16 = outp.tile([128, width], f16, tag=f"o16_{idx}")
            nc.vector.tensor_scalar(
                out=o16, in0=z_t[:, j, qsl],
                scalar1=acols[:, j:j + 1], scalar2=bcols[:, j:j + 1],
                op0=mybir.AluOpType.mult, op1=mybir.AluOpType.subtract)
            eng = nc.sync if idx % 2 == 0 else nc.scalar
            eng.dma_start(out=out_d[j * 128:(j + 1) * 128, qsl], in_=o16)


_NC_CACHE: dict = {}


def _get_nc():
    if "nc" not in _NC_CACHE:
        nc = bacc.Bacc(
            "TRN2",
            target_bir_lowering=False,
            debug=False,
            enable_asserts=True,
            num_devices=NCORES,
        )
        build_kernel(nc)
        nc.compile()
        _NC_CACHE["nc"] = nc
    return _NC_CACHE["nc"]


def _make_in_maps(inputs: dict) -> list[dict]:
    xi = np.asarray(inputs["xi"], np.float32).reshape(B, C, N)
    xj = np.asarray(inputs["xj"], np.float32).reshape(B, C, N)
    g_w = np.asarray(inputs["g_w"], np.float32)
    g_b = np.asarray(inputs["g_b"], np.float32)
    t_w = np.asarray(inputs["theta_w"], np.float32)
    t_b = np.asarray(inputs["theta_b"], np.float32)
    p_w = np.asarray(inputs["phi_w"], np.float32)
    p_b = np.asarray(inputs["phi_b"], np.float32)
    W_w = np.asarray(inputs["W_w"], np.float32)
    W_b = np.asarray(inputs["W_b"], np.float32)
    gam = np.asarray(inputs["bn_gamma"], np.float32)
    bet = np.asarray(inputs["bn_beta"], np.float32)

    def chunked(a):  # [256, F] -> [128, 2, F]
        return np.ascontiguousarray(a.reshape(2, 128, -1).transpose(1, 0, 2))

    # host-folded weight products (constant folding, fp32)
    Lp = W_w @ g_w                      # L' = W G   (device uses K/N)
    R = p_w.T @ t_w                     # R = P^T T
    wgb = W_w @ g_b
    b1 = t_w.T @ p_b
    ptb = p_w.T @ t_b
    c1 = float(p_b @ t_b)

    wm = np.zeros((128, NCH, WM_F), np.float16)
    wm[:, :, 0:C] = chunked(Lp.T)
    wm[:, :, C:2 * C] = chunked(R)
    wm[:, :, 2 * C] = ptb.reshape(2, 128).T
    aux = np.zeros((1, 2 * C + 8), np.float16)
    aux[0, 0:C] = b1.astype(np.float16)
    aux[0, C:2 * C] = wgb.astype(np.float16)
    aux[0, 2 * C] = np.float16(c1)
    sm = np.zeros((128, NCH, 3), np.float32)
    sm[:, :, 0] = gam.reshape(2, 128).T
    sm[:, :, 1] = bet.reshape(2, 128).T
    sm[:, :, 2] = W_b.reshape(2, 128).T

    def sw8(X):  # [C, N] -> fp8 SwInterleave weights [128, 16, 2, 129, 2]
        # w8[p, t, m, j, i] = X[m*128 + 127 - j, t*256 + i*128 + p]
        # (j < 128); w8[p, t, m, 128, i] = 1.0 (ones rhs column)
        Xr = X.reshape(2, 128, NT2, 2, 128)      # [m, c, t, i, p]
        w = Xr[:, ::-1].transpose(4, 2, 0, 1, 3)  # [p, t, m, j, i]
        o = np.ones((128, NT2, 2, 129, 2), np.float32)
        o[:, :, :, 0:128, :] = w
        return o.astype(ml_dtypes.float8_e4m3)

    in_maps = []
    for b in range(B):
        xib = chunked(xi[b]).astype(np.float16)      # [128,2,4096]
        in_maps.append({
            "xjw8": sw8(xj[b]), "xiw8": sw8(xi[b]), "xi": xib, "wm": wm,
            "aux": aux, "sm": sm,
        })
    return in_maps


def kernel(**inputs) -> np.ndarray:
    nc = _get_nc()
    in_maps = _make_in_maps(inputs)
    last_err = None
    for attempt in range(3):
        try:
            res = bass_utils.run_bass_kernel_spmd(
                nc, in_maps, core_ids=list(range(NCORES)),
            )
            break
        except Exception as e:  # transient device wedge: back off and retry
            last_err = e
            import time as _time
            _time.sleep(4.0 * (attempt + 1))
            try:
                import jax
                import jax.extend.backend as _jeb
                jax.clear_caches()
                _jeb.clear_backends()
            except Exception:
                pass
    else:
        raise last_err
    out = np.stack([res.results[c]["out"] for c in range(NCORES)])
    return np.ascontiguousarray(out.reshape(B, C, 64, 64).astype(np.float32))


if __name__ == "__main__":
    rng = np.random.default_rng(0)
    fake = {
        "xi": rng.standard_normal((B, C, 64, 64)).astype(np.float32),
        "xj": rng.standard_normal((B, C, 64, 64)).astype(np.float32),
        "g_w": (rng.standard_normal((C, C)) / 16).astype(np.float32),
        "g_b": (rng.standard_normal((C,)) / 16).astype(np.float32),
        "theta_w": (rng.standard_normal((C, C)) / 16).astype(np.float32),
        "theta_b": (rng.standard_normal((C,)) / 16).astype(np.float32),
        "phi_w": (rng.standard_normal((C, C)) / 16).astype(np.float32),
        "phi_b": (rng.standard_normal((C,)) / 16).astype(np.float32),
        "W_w": (rng.standard_normal((C, C)) / 16).astype(np.float32),
        "W_b": (rng.standard_normal((C,)) / 16).astype(np.float32),
        "bn_gamma": np.ones((C,), np.float32),
        "bn_beta": np.zeros((C,), np.float32),
    }
    out = kernel(**fake)
    print("out", out.shape, out.dtype, float(np.abs(out).mean()))


# revision 43
# speedup vs baseline: 1.0641x; 1.0080x over previous
"""Trainium2 Bass kernel for nn_DilatedContextAttentionModule (B=8, C=256, 64x64).

Reference, per batch element (N = 64*64 = 4096):
    g   = G xj + g_b 1^T;  th = T xi + t_b 1^T;  phi = P xj + p_b 1^T
    f   = th^T phi / N                      (N x N, linear -- NO softmax)
    y[c,n] = sum_m f[n,m] g[c,m]
    z   = W y + W_b 1^T + xi
    out = BatchNorm2d(z)                    (training-mode batch stats)

Algebraic collapse (Gram-matrix form; exact because f is linear):
    z = A xi + d 1^T,  A = I + E'
    E' = L'(K/N) R + a1 b1^T + wgb b2n^T,  K = xj xj^T  (C x C Gram)
    with host-folded L' = W G, R = P^T T, wgb = W g_b, b1 = T^T p_b,
    ptb = P^T t_b, c1 = p_b.t_b, and runtime vectors from sxj = xj @ 1:
    a1 = L'sxj/N + wgb,  b2n = R^T sxj/N,
    d = L'(K/N)ptb + c1 a1 + (sxj.ptb/N) wgb + W_b.

KEY STRUCTURE (vs the first 57.5us version):
 1. BN statistics computed ANALYTICALLY from a second Gram Kxi = xi xi^T:
        S1/N = A sxi/N + d
        S2/N = diag(A (Kxi/N) A^T) + d*(2 A sxi/N + d)
    so the cross-core ReduceScatter (cost-model floor ~15us) launches
    right after the two C x C Grams + cheap C x C algebra and overlaps
    the entire z-phase, instead of serializing after it.
 2. Both Grams run in fp8e4 with perf_mode=DoubleRowSwInterleave (the
    only DR mode walrus codegen accepts): 0.5 cycles/row with 256-deep
    contraction -> 4.2k PE cycles per Gram instead of 16.5k (f16).
    The fp8 tensor is shipped ONCE in the hardware's interleaved
    weights layout  w8[p,t,m,j,i] = X[m*128+127-j, t*256+i*128+p];
    the matmul rhs reads the SAME tile through a permuted AP view
    (p m j i -> p i m j), which yields Gram columns reversed within
    each 128-chunk; the PSUM->SBUF copy un-reverses with a stride -1
    AP at no cost.  Row sums (sxj, sxi) come from a separate fp8 ones
    rhs accumulated into a spare PSUM column.
    fp8 Gram noise measured end-to-end: rel err ~6.4e-3 (harness gate
    2e-2).
 3. Identity block of A is generated on device (affine_select), not
    shipped; weights DMA is 0.26 MB.
 4. Tail: RS -> ssum -> affine consts (Sqrt on ACT, rest DVE) ->
    out = a*z - b on DVE (f16 4x mode) in a small-first staircase with
    stores alternating SP/ACT queues.

Measured (TimelineSim with collectives, the harness metric): 44869 ns
vs the 57516 ns previous version; rel err vs fp32 jax on real HW
(8 axon NeuronCores): 6.44e-3 (harness gate 2e-2).
"""

import numpy as np
import ml_dtypes

import concourse.bass as bass
import concourse.bacc as bacc
import concourse.tile as tile
from concourse import mybir
from concourse import bass_utils

B = 8
C = 256
N = 4096          # 64 * 64
NCORES = 8
NCH = 2           # channel chunks of 128
NT2 = 16          # n chunks of 256 (fp8 DoubleRow Gram)
F32 = mybir.dt.float32
F16 = mybir.dt.float16
F8 = mybir.dt.float8e4
BN_EPS = 1e-5

# wmat layout (f16, [128, 2, 516]): per channel-chunk k:
#   [0:256] L'^T rows | [256:512] R rows | [512] ptb | pad
WM_LT = slice(0, 256)
WM_RC = slice(256, 512)
WM_F = 516


def build_kernel(nc, skip_cc: bool = False) -> None:
    f32, f16 = F32, F16
    xjw_d = nc.dram_tensor("xjw8", [128, NT2, 2, 129, 2], F8,
                           kind="ExternalInput").ap()
    xiw_d = nc.dram_tensor("xiw8", [128, NT2, 2, 129, 2], F8,
                           kind="ExternalInput").ap()
    xi_d = nc.dram_tensor("xi", [128, NCH, N], f16, kind="ExternalInput").ap()
    wm_d = nc.dram_tensor("wm", [128, NCH, WM_F], f16, kind="ExternalInput").ap()
    # aux row: [b1 (256) | wgb (256) | c1 (1) | pad]
    aux_d = nc.dram_tensor("aux", [1, 2 * C + 8], f16, kind="ExternalInput").ap()
    # f32 smalls: [gamma | beta | W_b] columns  -> [128, 2, 3]
    sm_d = nc.dram_tensor("sm", [128, NCH, 3], f32, kind="ExternalInput").ap()
    out_d = nc.dram_tensor("out", [C, N], f16, kind="ExternalOutput").ap()

    with tile.TileContext(nc) as tc:
        _body(tc, xjw_d, xiw_d, xi_d, wm_d, aux_d, sm_d, out_d, skip_cc=skip_cc)


def _body(tc, xjw_d, xiw_d, xi_d, wm_d, aux_d, sm_d, out_d,
          skip_cc: bool = False):
    nc = tc.nc
    f32, f16 = F32, F16
    import contextlib
    DR = mybir.MatmulPerfMode.DoubleRowSwInterleave

    with contextlib.ExitStack() as ctx:
        constp = ctx.enter_context(tc.tile_pool(name="const", bufs=1))
        datap = ctx.enter_context(tc.tile_pool(name="data", bufs=1))
        workp = ctx.enter_context(tc.tile_pool(name="work", bufs=4))
        rowsp = ctx.enter_context(tc.tile_pool(name="rows", bufs=2))
        outp = ctx.enter_context(tc.tile_pool(name="out", bufs=1))
        # PSUM budget (8 banks): psacc 2 + psg8 2 + psz 2 + psrow 1 + pscol 1
        psacc = ctx.enter_context(tc.tile_pool(name="ps_acc", bufs=2, space="PSUM"))
        psg8 = ctx.enter_context(tc.tile_pool(name="ps_g8", bufs=2, space="PSUM"))
        psz = ctx.enter_context(tc.tile_pool(name="ps_z", bufs=2, space="PSUM"))
        psrow = ctx.enter_context(tc.tile_pool(name="ps_row", bufs=1, space="PSUM"))
        pscol = ctx.enter_context(tc.tile_pool(name="ps_col", bufs=1, space="PSUM"))
        dramp = ctx.enter_context(tc.tile_pool(name="dram", bufs=2, space="DRAM"))

        # ---- loads, all on the SP queue so the single DMA device ------
        # ---- services them in exactly this priority order -------------
        xjw = datap.tile([128, NT2, 2, 129, 2], F8, tag="xjw")
        NXJ = 2
        for h in range(NXJ):
            sl = slice(h * (NT2 // NXJ), (h + 1) * (NT2 // NXJ))
            nc.sync.dma_start(out=xjw[:, sl], in_=xjw_d[:, sl])
        wm = constp.tile([128, NCH, WM_F], f16, tag="wm")
        nc.sync.dma_start(out=wm, in_=wm_d)
        aux = constp.tile([1, 2 * C + 8], f16, tag="aux")
        nc.sync.dma_start(out=aux, in_=aux_d)
        sm = constp.tile([128, NCH, 3], f32, tag="sm")
        nc.sync.dma_start(out=sm, in_=sm_d)
        xiw = datap.tile([128, NT2, 2, 129, 2], F8, tag="xiw")
        NXI8 = 4
        for h in range(NXI8):
            sl = slice(h * (NT2 // NXI8), (h + 1) * (NT2 // NXI8))
            nc.sync.dma_start(out=xiw[:, sl], in_=xiw_d[:, sl])
        xi_t = datap.tile([128, NCH, N], f16, tag="xi")
        NXI = 4
        for h in range(NXI):
            sl = slice(h * (N // NXI), (h + 1) * (N // NXI))
            nc.sync.dma_start(out=xi_t[:, :, sl], in_=xi_d[:, :, sl])

        eps = constp.tile([128, 1], f32, tag="eps")
        nc.vector.memset(eps, BN_EPS)
        ones1 = constp.tile([128, 1], f16, tag="ones1")
        nc.vector.memset(ones1, 1.0)
        # identity rows, generated on device: ident[:, m, c] = 1 iff
        # c == m*128 + p
        ones256 = constp.tile([128, C], f16, tag="ones256")
        nc.vector.memset(ones256, 1.0)
        ident = constp.tile([128, NCH, C], f16, tag="ident")
        for m in range(NCH):
            nc.gpsimd.affine_select(
                out=ident[:, m, :], in_=ones256, pattern=[[1, C]],
                compare_op=mybir.AluOpType.is_equal, fill=0.0,
                base=-m * 128, channel_multiplier=-1)

        def lt(k, csl=slice(0, C)):
            return wm[:, k, WM_LT][:, csl]

        def rcw(k, csl=slice(0, C)):
            return wm[:, k, WM_RC][:, csl]

        def ptbc(k):
            return wm[:, k, 512:513]

        # ---- PE warm-up: hold the p-state at full clock until the ----
        # ---- first Gram chunk arrives (cold PE runs at 0.65 GHz)  ----
        warm = constp.tile([128, 640], f16, tag="warm")
        nc.gpsimd.memset(warm, 0.0)
        wps = psz.tile([128, 512], f32, tag="zt", name="warm_ps")
        nc.tensor.matmul(wps[:, 0:128], warm[:, 0:128], warm[:, 128:256],
                         start=True, stop=True)
        NWARM = 6
        for w in range(NWARM):
            nc.tensor.matmul(wps, warm[:, 0:128], warm[:, 128:640],
                             start=True, stop=True)

        def gram(ps_tiles, src):
            """fp8 DoubleRowSwInterleave Gram: src is the interleaved
            weights tile [128, NT2, 2, 129, 2] whose j=128 slot holds
            ones, so each matmul's 258-wide output carries the row sums
            at block-relative column 128 inside the SAME accumulation
            group (PSUM zero regions allow one group per bank)."""
            for t in range(NT2):
                rhs = src[:, t].rearrange("p m j i -> p i m j")
                for m in range(NCH):
                    nc.tensor.matmul(
                        ps_tiles[m][:, 0:258], src[:, t, m, 0:128, :], rhs,
                        start=(t == 0), stop=(t == NT2 - 1), perf_mode=DR)

        def gram_copy(ps_tiles, out_tiles, tagp):
            """PSUM -> SBUF f16 with 1/N scale; un-reverses the
            within-chunk column order (psum col = mb*129 + 127 - c,
            row sums at mb*129 + 128); row-sum column copied FIRST so
            consumers of sxn can start early."""
            for m in range(NCH):
                t = workp.tile([128, C + 1], f16, tag=f"{tagp}{m}")
                blk = ps_tiles[m][:, 0:258].rearrange(
                    "p (mb c) -> p mb c", mb=NCH)
                rev = blk[:, :, 0:128][:, :, ::-1]
                if m == 0:
                    nc.scalar.activation(
                        out=t[:, C:C + 1], in_=blk[:, 0, 128:129],
                        func=mybir.ActivationFunctionType.Identity,
                        scale=1.0 / N)
                    nc.scalar.activation(
                        out=t[:, 0:C], in_=rev,
                        func=mybir.ActivationFunctionType.Identity,
                        scale=1.0 / N)
                else:
                    nc.vector.tensor_scalar_mul(
                        t[:, C:C + 1], blk[:, 0, 128:129], 1.0 / N)
                    nc.vector.tensor_scalar_mul(t[:, 0:C], rev, 1.0 / N)
                out_tiles.append(t)

        # ---- phase A: K_aug = [xj|1]^T [xj|1] -> K and sxj ------------
        K_ps = [psacc.tile([128, C + 8], f32, tag="acc", name=f"K_ps{m}")
                for m in range(NCH)]
        gram(K_ps, xjw)
        K_sb = []
        gram_copy(K_ps, K_sb, "K")

        def sxn(k):  # sxj/N column (f16)
            return K_sb[k][:, C:C + 1]

        # ---- phases B/C/D: C x C algebra — high priority so the ------
        # ---- scheduler orders these PE ops ahead of the Kxi Gram -----
        # ---- matmuls (whose xiw DMA waits would head-of-line block ---
        # ---- the in-order PE sequencer) ------------------------------
        hp_ctx = tc.high_priority()
        hp_ctx.__enter__()

        # phase B: runtime rows a1 = L'sxj/N + wgb, b2n = R^T sxj/N
        rows_ps = psrow.tile([1, 2 * C], f32, tag="rows")
        for k in range(NCH):
            nc.tensor.matmul(rows_ps[:, 0:C], sxn(k), lt(k),
                             start=(k == 0), stop=(k == NCH - 1))
        for k in range(NCH):
            nc.tensor.matmul(rows_ps[:, C:2 * C], sxn(k), rcw(k),
                             start=(k == 0), stop=(k == NCH - 1))
        a1row = rowsp.tile([1, C], f16, tag="a1row")
        nc.vector.tensor_add(a1row, rows_ps[:, 0:C], aux[:, C:2 * C])
        b2row = rowsp.tile([1, C], f16, tag="b2row")
        nc.vector.tensor_copy(b2row, rows_ps[:, C:2 * C])

        # phase C: T1 = (K/N) L'^T;  ET = R^T T1 + rank1 + I
        T1_sb = []
        for cb in range(NCH):
            t1_ps = psacc.tile([128, C], f32, tag="acc")
            csl = slice(cb * 128, (cb + 1) * 128)
            for jb in range(NCH):
                nc.tensor.matmul(t1_ps, K_sb[jb][:, csl], lt(jb),
                                 start=(jb == 0), stop=(jb == NCH - 1))
            t = workp.tile([128, C], f16, tag=f"T1{cb}")
            if cb == 0:
                nc.scalar.copy(t, t1_ps)
            else:
                nc.vector.tensor_copy(t, t1_ps)
            T1_sb.append(t)
        ET_sb = []
        for ob in range(NCH):
            et_ps = psacc.tile([128, C], f32, tag="acc")
            osl = slice(ob * 128, (ob + 1) * 128)
            for cb in range(NCH):
                nc.tensor.matmul(et_ps, rcw(cb, osl), T1_sb[cb],
                                 start=(cb == 0), stop=False)
            nc.tensor.matmul(et_ps, aux[:, osl], a1row, start=False, stop=False)
            nc.tensor.matmul(et_ps, b2row[:, osl], aux[:, C:2 * C],
                             start=False, stop=False)
            # += I_block via matmul (keeps DVE off the critical chain)
            nc.tensor.matmul(et_ps, ident[:, 0, 0:128], ident[:, ob, :],
                             start=False, stop=True)
            t = workp.tile([128, C], f16, tag=f"ET{ob}")
            if ob == 0:
                nc.scalar.copy(t, et_ps)
            else:
                nc.vector.tensor_copy(t, et_ps)
            ET_sb.append(t)

        # phase D: d = L'(K/N)ptb + c1 a1 + c2n wgb + W_b
        col_ps = pscol.tile([128, 12], f32, tag="cols")
        for cb in range(NCH):
            csl = slice(cb * 128, (cb + 1) * 128)
            for jb in range(NCH):
                nc.tensor.matmul(col_ps[:, cb:cb + 1], K_sb[jb][:, csl],
                                 ptbc(jb),
                                 start=(jb == 0), stop=(jb == NCH - 1))
        # c2n = (sxj.ptb)/N via the sxj/N column of K_sb (1-partition out)
        for jb in range(NCH):
            nc.tensor.matmul(col_ps[0:1, 6:7], sxn(jb), ptbc(jb),
                             start=(jb == 0), stop=(jb == NCH - 1))
        kpc = rowsp.tile([128, NCH], f16, tag="kpc")
        nc.scalar.copy(kpc, col_ps[:, 0:NCH])
        c2cell = rowsp.tile([1, 1], f16, tag="c2cell")
        nc.vector.tensor_copy(c2cell, col_ps[0:1, 6:7])
        for ob in range(NCH):
            osl = slice(ob * 128, (ob + 1) * 128)
            for cb in range(NCH):
                nc.tensor.matmul(col_ps[:, 2 + ob:3 + ob], lt(cb, osl),
                                 kpc[:, cb:cb + 1],
                                 start=(cb == 0), stop=False)
            nc.tensor.matmul(col_ps[:, 2 + ob:3 + ob], a1row[:, osl],
                             aux[:, 2 * C:2 * C + 1], start=False, stop=False)
            nc.tensor.matmul(col_ps[:, 2 + ob:3 + ob],
                             aux[:, C + ob * 128:C + (ob + 1) * 128],
                             c2cell, start=False, stop=True)
        dcol = rowsp.tile([128, NCH], f32, tag="dcol")
        nc.vector.tensor_add(dcol, col_ps[:, 2:2 + NCH], sm[:, :, 2])
        hp_ctx.__exit__(None, None, None)

        # ---- phase S: analytic BN stats ------------------------------
        K2_ps = [psg8.tile([128, C + 8], f32, tag="g8", name=f"K2_ps{m}")
                 for m in range(NCH)]
        gram(K2_ps, xiw)
        K2_sb = []
        gram_copy(K2_ps, K2_sb, "K2")

        def sxin(k):  # sxi/N column (f16)
            return K2_sb[k][:, C:C + 1]

        # QT[j, c] = (Kxi/N A^T)[j, c]; P = QT .* ET; diag = sum_j P
        P_sb = []
        for jb in range(NCH):
            qt_ps = psacc.tile([128, C], f32, tag="acc")
            jsl = slice(jb * 128, (jb + 1) * 128)
            for kb in range(NCH):
                nc.tensor.matmul(qt_ps, K2_sb[kb][:, jsl], ET_sb[kb],
                                 start=(kb == 0), stop=(kb == NCH - 1))
            p = workp.tile([128, C], f16, tag=f"P{jb}")
            nc.vector.tensor_mul(p, qt_ps, ET_sb[jb])
            P_sb.append(p)
        # col_ps cols 8-9: diag(A Kxi/N A^T); cols 10-11: asx = A sxi/N
        for cb in range(NCH):
            csl = slice(cb * 128, (cb + 1) * 128)
            for jb in range(NCH):
                nc.tensor.matmul(col_ps[:, 8 + cb:9 + cb], P_sb[jb][:, csl],
                                 ones1, start=(jb == 0), stop=(jb == NCH - 1))
            for kb in range(NCH):
                nc.tensor.matmul(col_ps[:, 10 + cb:11 + cb],
                                 ET_sb[kb][:, csl], sxin(kb),
                                 start=(kb == 0), stop=(kb == NCH - 1))
        xsb = rowsp.tile([128, 4], f32, tag="xsb")
        nc.vector.tensor_copy(xsb, col_ps[:, 8:12])

        # spack = [S1n_0 | S1n_1 | S2n_0 | S2n_1]  (per-core mean/meansq)
        spack = rowsp.tile([128, 4], f32, tag="spack")
        tmp2 = rowsp.tile([128, NCH], f32, tag="tmp2")
        for cb in range(NCH):
            dc = dcol[:, cb:cb + 1]
            asxc = xsb[:, 2 + cb:3 + cb]
            nc.vector.tensor_scalar(
                out=spack[:, cb:cb + 1], in0=asxc, scalar1=dc, scalar2=0.0,
                op0=mybir.AluOpType.add, op1=mybir.AluOpType.add)
            nc.vector.tensor_scalar(
                out=tmp2[:, cb:cb + 1], in0=asxc, scalar1=2.0, scalar2=dc,
                op0=mybir.AluOpType.mult, op1=mybir.AluOpType.add)
            nc.vector.tensor_scalar(
                out=spack[:, 2 + cb:3 + cb], in0=tmp2[:, cb:cb + 1],
                scalar1=dc, scalar2=xsb[:, cb:cb + 1],
                op0=mybir.AluOpType.mult, op1=mybir.AluOpType.add)

        # ---- ONE ReduceScatter: input = own stats tiled 8x, so every --
        # ---- core's scattered block is already the full global sum ----
        cc_in = dramp.tile([NCORES * 128, 4], f32, tag="cc_in", name="cc_in")
        cc_out = dramp.tile([128, 4], f32, tag="cc_out", name="cc_out")
        nc.sync.dma_start(
            out=cc_in.rearrange("(r p) f -> p r f", p=128),
            in_=spack.unsqueeze(1).broadcast_to([128, NCORES, 4]))
        if skip_cc:
            nc.sync.dma_start(out=cc_out, in_=cc_in[0:128, :])
        else:
            nc.gpsimd.collective_compute(
                "ReduceScatter",
                mybir.AluOpType.add,
                replica_groups=[list(range(NCORES))],
                ins=[cc_in.opt()],
                outs=[cc_out.opt()],
            )
        ssum = rowsp.tile([128, 4], f32, tag="ssum")
        nc.sync.dma_start(out=ssum, in_=cc_out)

        # ---- phase Z: z = A xi (+d via ACT) -> z_t (f16) --------------
        z_t = datap.tile([128, NCH, N], f16, tag="z")
        NZT = 8
        for tp in range(NZT):
            tsl = slice(tp * 512, (tp + 1) * 512)
            for j in range(NCH):
                jsl = slice(j * 128, (j + 1) * 128)
                z_ps = psz.tile([128, 512], f32, tag="zt")
                for k in range(NCH):
                    nc.tensor.matmul(
                        z_ps, ET_sb[k][:, jsl], xi_t[:, k, tsl],
                        start=(k == 0), stop=(k == NCH - 1))
                nc.scalar.activation(
                    out=z_t[:, j, tsl], in_=z_ps,
                    func=mybir.ActivationFunctionType.Identity,
                    bias=dcol[:, j:j + 1], scale=1.0)

        # ---- affine constants: a = gamma*rsqrt(var+eps); b = mean*a-beta
        # ssum = (8*mean | 8*meansq) per chunk; fold the /8 into the ops.
        t2 = rowsp.tile([128, NCH], f32, tag="t2")
        nc.vector.tensor_mul(t2, ssum[:, 0:2], ssum[:, 0:2])
        v8 = rowsp.tile([128, NCH], f32, tag="v8")
        nc.vector.scalar_tensor_tensor(
            out=v8, in0=t2, scalar=-1.0 / NCORES, in1=ssum[:, 2:4],
            op0=mybir.AluOpType.mult, op1=mybir.AluOpType.add)
        scol = rowsp.tile([128, NCH], f32, tag="scol")
        nc.scalar.activation(
            out=scol, in_=v8, func=mybir.ActivationFunctionType.Sqrt,
            bias=eps, scale=1.0 / NCORES)
        acols = rowsp.tile([128, NCH], f32, tag="acols")
        nc.vector.reciprocal(out=acols, in_=scol)
        nc.vector.tensor_mul(acols, acols, sm[:, :, 0])
        bcols = rowsp.tile([128, NCH], f32, tag="bcols")
        nc.vector.scalar_tensor_tensor(
            out=bcols, in0=ssum[:, 0:2], scalar=1.0 / NCORES, in1=acols,
            op0=mybir.AluOpType.mult, op1=mybir.AluOpType.mult)
        nc.vector.scalar_tensor_tensor(
            out=bcols, in0=bcols, scalar=1.0, in1=sm[:, :, 1],
            op0=mybir.AluOpType.mult, op1=mybir.AluOpType.subtract)

        # ---- apply out = a*z - b on DVE (f16 4x mode), staircased -----
        PIECES = [(0, 0, 1024), (1, 0, 1024), (0, 1024, 1536),
                  (1, 1024, 1536), (0, 2560, 1536), (1, 2560, 1536)]
        for idx, (j, off, width) in enumerate(PIECES):
            qsl = slice(off, off + width)
            o16 = outp.tile([128, width], f16, tag=f"o16_{idx}")
            nc.vector.tensor_scalar(
                out=o16, in0=z_t[:, j, qsl],
                scalar1=acols[:, j:j + 1], scalar2=bcols[:, j:j + 1],
                op0=mybir.AluOpType.mult, op1=mybir.AluOpType.subtract)
            eng = nc.sync if idx % 2 == 0 else nc.scalar
            eng.dma_start(out=out_d[j * 128:(j + 1) * 128, qsl], in_=o16)


_NC_CACHE: dict = {}


def _get_nc():
    if "nc" not in _NC_CACHE:
        nc = bacc.Bacc(
            "TRN2",
            target_bir_lowering=False,
            debug=False,
            enable_asserts=True,
            num_devices=NCORES,
        )
        build_kernel(nc)
        nc.compile()
        _NC_CACHE["nc"] = nc
    return _NC_CACHE["nc"]


def _make_in_maps(inputs: dict) -> list[dict]:
    xi = np.asarray(inputs["xi"], np.float32).reshape(B, C, N)
    xj = np.asarray(inputs["xj"], np.float32).reshape(B, C, N)
    g_w = np.asarray(inputs["g_w"], np.float32)
    g_b = np.asarray(inputs["g_b"], np.float32)
    t_w = np.asarray(inputs["theta_w"], np.float32)
    t_b = np.asarray(inputs["theta_b"], np.float32)
    p_w = np.asarray(inputs["phi_w"], np.float32)
    p_b = np.asarray(inputs["phi_b"], np.float32)
    W_w = np.asarray(inputs["W_w"], np.float32)
    W_b = np.asarray(inputs["W_b"], np.float32)
    gam = np.asarray(inputs["bn_gamma"], np.float32)
    bet = np.asarray(inputs["bn_beta"], np.float32)

    def chunked(a):  # [256, F] -> [128, 2, F]
        return np.ascontiguousarray(a.reshape(2, 128, -1).transpose(1, 0, 2))

    # host-folded weight products (constant folding, fp32)
    Lp = W_w @ g_w                      # L' = W G   (device uses K/N)
    R = p_w.T @ t_w                     # R = P^T T
    wgb = W_w @ g_b
    b1 = t_w.T @ p_b
    ptb = p_w.T @ t_b
    c1 = float(p_b @ t_b)

    wm = np.zeros((128, NCH, WM_F), np.float16)
    wm[:, :, 0:C] = chunked(Lp.T)
    wm[:, :, C:2 * C] = chunked(R)
    wm[:, :, 2 * C] = ptb.reshape(2, 128).T
    aux = np.zeros((1, 2 * C + 8), np.float16)
    aux[0, 0:C] = b1.astype(np.float16)
    aux[0, C:2 * C] = wgb.astype(np.float16)
    aux[0, 2 * C] = np.float16(c1)
    sm = np.zeros((128, NCH, 3), np.float32)
    sm[:, :, 0] = gam.reshape(2, 128).T
    sm[:, :, 1] = bet.reshape(2, 128).T
    sm[:, :, 2] = W_b.reshape(2, 128).T

    def sw8(X):  # [C, N] -> fp8 SwInterleave weights [128, 16, 2, 129, 2]
        # w8[p, t, m, j, i] = X[m*128 + 127 - j, t*256 + i*128 + p]
        # (j < 128); w8[p, t, m, 128, i] = 1.0 (ones rhs column)
        Xr = X.reshape(2, 128, NT2, 2, 128)      # [m, c, t, i, p]
        w = Xr[:, ::-1].transpose(4, 2, 0, 1, 3)  # [p, t, m, j, i]
        o = np.ones((128, NT2, 2, 129, 2), np.float32)
        o[:, :, :, 0:128, :] = w
        return o.astype(ml_dtypes.float8_e4m3)

    in_maps = []
    for b in range(B):
        xib = chunked(xi[b]).astype(np.float16)      # [128,2,4096]
        in_maps.append({
            "xjw8": sw8(xj[b]), "xiw8": sw8(xi[b]), "xi": xib, "wm": wm,
            "aux": aux, "sm": sm,
        })
    return in_maps


def kernel(**inputs) -> np.ndarray:
    nc = _get_nc()
    in_maps = _make_in_maps(inputs)
    last_err = None
    for attempt in range(3):
        try:
            res = bass_utils.run_bass_kernel_spmd(
                nc, in_maps, core_ids=list(range(NCORES)),
            )
            break
        except Exception as e:  # transient device wedge: back off and retry
            last_err = e
            import time as _time
            _time.sleep(4.0 * (attempt + 1))
            try:
                import jax
                import jax.extend.backend as _jeb
                jax.clear_caches()
                _jeb.clear_backends()
            except Exception:
                pass
    else:
        raise last_err
    out = np.stack([res.results[c]["out"] for c in range(NCORES)])
    return np.ascontiguousarray(out.reshape(B, C, 64, 64).astype(np.float32))


if __name__ == "__main__":
    rng = np.random.default_rng(0)
    fake = {
        "xi": rng.standard_normal((B, C, 64, 64)).astype(np.float32),
        "xj": rng.standard_normal((B, C, 64, 64)).astype(np.float32),
        "g_w": (rng.standard_normal((C, C)) / 16).astype(np.float32),
        "g_b": (rng.standard_normal((C,)) / 16).astype(np.float32),
        "theta_w": (rng.standard_normal((C, C)) / 16).astype(np.float32),
        "theta_b": (rng.standard_normal((C,)) / 16).astype(np.float32),
        "phi_w": (rng.standard_normal((C, C)) / 16).astype(np.float32),
        "phi_b": (rng.standard_normal((C,)) / 16).astype(np.float32),
        "W_w": (rng.standard_normal((C, C)) / 16).astype(np.float32),
        "W_b": (rng.standard_normal((C,)) / 16).astype(np.float32),
        "bn_gamma": np.ones((C,), np.float32),
        "bn_beta": np.zeros((C,), np.float32),
    }
    out = kernel(**fake)
    print("out", out.shape, out.dtype, float(np.abs(out).mean()))


# revision 44
# speedup vs baseline: 1.0685x; 1.0041x over previous
"""Trainium2 Bass kernel for nn_DilatedContextAttentionModule (B=8, C=256, 64x64).

Reference, per batch element (N = 64*64 = 4096):
    g   = G xj + g_b 1^T;  th = T xi + t_b 1^T;  phi = P xj + p_b 1^T
    f   = th^T phi / N                      (N x N, linear -- NO softmax)
    y[c,n] = sum_m f[n,m] g[c,m]
    z   = W y + W_b 1^T + xi
    out = BatchNorm2d(z)                    (training-mode batch stats)

Algebraic collapse (Gram-matrix form; exact because f is linear):
    z = A xi + d 1^T,  A = I + E'
    E' = L'(K/N) R + a1 b1^T + wgb b2n^T,  K = xj xj^T  (C x C Gram)
    with host-folded L' = W G, R = P^T T, wgb = W g_b, b1 = T^T p_b,
    ptb = P^T t_b, c1 = p_b.t_b, and runtime vectors from sxj = xj @ 1:
    a1 = L'sxj/N + wgb,  b2n = R^T sxj/N,
    d = L'(K/N)ptb + c1 a1 + (sxj.ptb/N) wgb + W_b.

KEY STRUCTURE (vs the first 57.5us version):
 1. BN statistics computed ANALYTICALLY from a second Gram Kxi = xi xi^T:
        S1/N = A sxi/N + d
        S2/N = diag(A (Kxi/N) A^T) + d*(2 A sxi/N + d)
    so the cross-core ReduceScatter (cost-model floor ~15us) launches
    right after the two C x C Grams + cheap C x C algebra and overlaps
    the entire z-phase, instead of serializing after it.
 2. Both Grams run in fp8e4 with perf_mode=DoubleRowSwInterleave (the
    only DR mode walrus codegen accepts): 0.5 cycles/row with 256-deep
    contraction -> 4.2k PE cycles per Gram instead of 16.5k (f16).
    The fp8 tensor is shipped ONCE in the hardware's interleaved
    weights layout  w8[p,t,m,j,i] = X[m*128+127-j, t*256+i*128+p];
    the matmul rhs reads the SAME tile through a permuted AP view
    (p m j i -> p i m j), which yields Gram columns reversed within
    each 128-chunk; the PSUM->SBUF copy un-reverses with a stride -1
    AP at no cost.  Row sums (sxj, sxi) come from a separate fp8 ones
    rhs accumulated into a spare PSUM column.
    fp8 Gram noise measured end-to-end: rel err ~6.4e-3 (harness gate
    2e-2).
 3. Identity block of A is generated on device (affine_select), not
    shipped; weights DMA is 0.26 MB.
 4. Tail: RS -> ssum -> affine consts (Sqrt on ACT, rest DVE) ->
    out = a*z - b on DVE (f16 4x mode) in a small-first staircase with
    stores alternating SP/ACT queues.

Measured (TimelineSim with collectives, the harness metric): 44869 ns
vs the 57516 ns previous version; rel err vs fp32 jax on real HW
(8 axon NeuronCores): 6.44e-3 (harness gate 2e-2).
"""

import numpy as np
import ml_dtypes

import concourse.bass as bass
import concourse.bacc as bacc
import concourse.tile as tile
from concourse import mybir
from concourse import bass_utils

B = 8
C = 256
N = 4096          # 64 * 64
NCORES = 8
NCH = 2           # channel chunks of 128
NT2 = 16          # n chunks of 256 (fp8 DoubleRow Gram)
F32 = mybir.dt.float32
F16 = mybir.dt.float16
F8 = mybir.dt.float8e4
BN_EPS = 1e-5

# wmat layout (f16, [128, 2, 516]): per channel-chunk k:
#   [0:256] L'^T rows | [256:512] R rows | [512] ptb | pad
WM_LT = slice(0, 256)
WM_RC = slice(256, 512)
WM_F = 516


def build_kernel(nc, skip_cc: bool = False) -> None:
    f32, f16 = F32, F16
    xjw_d = nc.dram_tensor("xjw8", [128, NT2, 2, 129, 2], F8,
                           kind="ExternalInput").ap()
    xiw_d = nc.dram_tensor("xiw8", [128, NT2, 2, 129, 2], F8,
                           kind="ExternalInput").ap()
    xi_d = nc.dram_tensor("xi", [128, NCH, N], f16, kind="ExternalInput").ap()
    wm_d = nc.dram_tensor("wm", [128, NCH, WM_F], f16, kind="ExternalInput").ap()
    # aux row: [b1 (256) | wgb (256) | c1 (1) | pad]
    aux_d = nc.dram_tensor("aux", [1, 2 * C + 8], f16, kind="ExternalInput").ap()
    # f32 smalls: [gamma | beta | W_b] columns  -> [128, 2, 3]
    sm_d = nc.dram_tensor("sm", [128, NCH, 3], f32, kind="ExternalInput").ap()
    out_d = nc.dram_tensor("out", [C, N], f16, kind="ExternalOutput").ap()

    with tile.TileContext(nc) as tc:
        _body(tc, xjw_d, xiw_d, xi_d, wm_d, aux_d, sm_d, out_d, skip_cc=skip_cc)


def _body(tc, xjw_d, xiw_d, xi_d, wm_d, aux_d, sm_d, out_d,
          skip_cc: bool = False):
    nc = tc.nc
    f32, f16 = F32, F16
    import contextlib
    DR = mybir.MatmulPerfMode.DoubleRowSwInterleave

    with contextlib.ExitStack() as ctx:
        constp = ctx.enter_context(tc.tile_pool(name="const", bufs=1))
        datap = ctx.enter_context(tc.tile_pool(name="data", bufs=1))
        workp = ctx.enter_context(tc.tile_pool(name="work", bufs=4))
        rowsp = ctx.enter_context(tc.tile_pool(name="rows", bufs=2))
        outp = ctx.enter_context(tc.tile_pool(name="out", bufs=1))
        # PSUM budget (8 banks): psacc 2 + psg8 2 + psz 2 + psrow 1 + pscol 1
        psacc = ctx.enter_context(tc.tile_pool(name="ps_acc", bufs=2, space="PSUM"))
        psg8 = ctx.enter_context(tc.tile_pool(name="ps_g8", bufs=2, space="PSUM"))
        psz = ctx.enter_context(tc.tile_pool(name="ps_z", bufs=2, space="PSUM"))
        psrow = ctx.enter_context(tc.tile_pool(name="ps_row", bufs=1, space="PSUM"))
        pscol = ctx.enter_context(tc.tile_pool(name="ps_col", bufs=1, space="PSUM"))
        dramp = ctx.enter_context(tc.tile_pool(name="dram", bufs=2, space="DRAM"))

        # ---- loads, all on the SP queue so the single DMA device ------
        # ---- services them in exactly this priority order -------------
        xjw = datap.tile([128, NT2, 2, 129, 2], F8, tag="xjw")
        NXJ = 2
        for h in range(NXJ):
            sl = slice(h * (NT2 // NXJ), (h + 1) * (NT2 // NXJ))
            nc.sync.dma_start(out=xjw[:, sl], in_=xjw_d[:, sl])
        wm = constp.tile([128, NCH, WM_F], f16, tag="wm")
        nc.sync.dma_start(out=wm, in_=wm_d)
        aux = constp.tile([1, 2 * C + 8], f16, tag="aux")
        nc.sync.dma_start(out=aux, in_=aux_d)
        sm = constp.tile([128, NCH, 3], f32, tag="sm")
        nc.sync.dma_start(out=sm, in_=sm_d)
        xiw = datap.tile([128, NT2, 2, 129, 2], F8, tag="xiw")
        NXI8 = 4
        for h in range(NXI8):
            sl = slice(h * (NT2 // NXI8), (h + 1) * (NT2 // NXI8))
            nc.sync.dma_start(out=xiw[:, sl], in_=xiw_d[:, sl])
        xi_t = datap.tile([128, NCH, N], f16, tag="xi")
        NXI = 4
        for h in range(NXI):
            sl = slice(h * (N // NXI), (h + 1) * (N // NXI))
            nc.sync.dma_start(out=xi_t[:, :, sl], in_=xi_d[:, :, sl])

        eps = constp.tile([128, 1], f32, tag="eps")
        nc.vector.memset(eps, BN_EPS)
        ones1 = constp.tile([128, 1], f16, tag="ones1")
        nc.vector.memset(ones1, 1.0)
        # identity rows, generated on device: ident[:, m, c] = 1 iff
        # c == m*128 + p
        ones256 = constp.tile([128, C], f16, tag="ones256")
        nc.vector.memset(ones256, 1.0)
        ident = constp.tile([128, NCH, C], f16, tag="ident")
        for m in range(NCH):
            nc.gpsimd.affine_select(
                out=ident[:, m, :], in_=ones256, pattern=[[1, C]],
                compare_op=mybir.AluOpType.is_equal, fill=0.0,
                base=-m * 128, channel_multiplier=-1)

        def lt(k, csl=slice(0, C)):
            return wm[:, k, WM_LT][:, csl]

        def rcw(k, csl=slice(0, C)):
            return wm[:, k, WM_RC][:, csl]

        def ptbc(k):
            return wm[:, k, 512:513]

        # ---- PE warm-up: hold the p-state at full clock until the ----
        # ---- first Gram chunk arrives (cold PE runs at 0.65 GHz)  ----
        warm = constp.tile([128, 640], f16, tag="warm")
        nc.gpsimd.memset(warm, 0.0)
        wps = psz.tile([128, 512], f32, tag="zt", name="warm_ps")
        nc.tensor.matmul(wps[:, 0:128], warm[:, 0:128], warm[:, 128:256],
                         start=True, stop=True)
        NWARM = 6
        for w in range(NWARM):
            nc.tensor.matmul(wps, warm[:, 0:128], warm[:, 128:640],
                             start=True, stop=True)

        def gram(ps_tiles, src):
            """fp8 DoubleRowSwInterleave Gram: src is the interleaved
            weights tile [128, NT2, 2, 129, 2] whose j=128 slot holds
            ones, so each matmul's 258-wide output carries the row sums
            at block-relative column 128 inside the SAME accumulation
            group (PSUM zero regions allow one group per bank)."""
            for t in range(NT2):
                rhs = src[:, t].rearrange("p m j i -> p i m j")
                for m in range(NCH):
                    nc.tensor.matmul(
                        ps_tiles[m][:, 0:258], src[:, t, m, 0:128, :], rhs,
                        start=(t == 0), stop=(t == NT2 - 1), perf_mode=DR)

        def gram_copy(ps_tiles, out_tiles, tagp):
            """PSUM -> SBUF f16 with 1/N scale; un-reverses the
            within-chunk column order (psum col = mb*129 + 127 - c,
            row sums at mb*129 + 128); row-sum column copied FIRST so
            consumers of sxn can start early."""
            for m in range(NCH):
                t = workp.tile([128, C + 1], f16, tag=f"{tagp}{m}")
                blk = ps_tiles[m][:, 0:258].rearrange(
                    "p (mb c) -> p mb c", mb=NCH)
                rev = blk[:, :, 0:128][:, :, ::-1]
                if m == 0:
                    nc.scalar.activation(
                        out=t[:, C:C + 1], in_=blk[:, 0, 128:129],
                        func=mybir.ActivationFunctionType.Identity,
                        scale=1.0 / N)
                    nc.scalar.activation(
                        out=t[:, 0:C], in_=rev,
                        func=mybir.ActivationFunctionType.Identity,
                        scale=1.0 / N)
                else:
                    nc.vector.tensor_scalar_mul(
                        t[:, C:C + 1], blk[:, 0, 128:129], 1.0 / N)
                    nc.vector.tensor_scalar_mul(t[:, 0:C], rev, 1.0 / N)
                out_tiles.append(t)

        # ---- phase A: K_aug = [xj|1]^T [xj|1] -> K and sxj ------------
        K_ps = [psacc.tile([128, C + 8], f32, tag="acc", name=f"K_ps{m}")
                for m in range(NCH)]
        gram(K_ps, xjw)
        K_sb = []
        gram_copy(K_ps, K_sb, "K")

        def sxn(k):  # sxj/N column (f16)
            return K_sb[k][:, C:C + 1]

        # ---- phases B/C/D: C x C algebra — high priority so the ------
        # ---- scheduler orders these PE ops ahead of the Kxi Gram -----
        # ---- matmuls (whose xiw DMA waits would head-of-line block ---
        # ---- the in-order PE sequencer) ------------------------------
        hp_ctx = tc.high_priority()
        hp_ctx.__enter__()

        # phase B: runtime rows a1 = L'sxj/N + wgb, b2n = R^T sxj/N
        rows_ps = psrow.tile([1, 2 * C], f32, tag="rows")
        for k in range(NCH):
            nc.tensor.matmul(rows_ps[:, 0:C], sxn(k), lt(k),
                             start=(k == 0), stop=(k == NCH - 1))
        for k in range(NCH):
            nc.tensor.matmul(rows_ps[:, C:2 * C], sxn(k), rcw(k),
                             start=(k == 0), stop=(k == NCH - 1))
        a1row = rowsp.tile([1, C], f16, tag="a1row")
        nc.vector.tensor_add(a1row, rows_ps[:, 0:C], aux[:, C:2 * C])
        b2row = rowsp.tile([1, C], f16, tag="b2row")
        nc.vector.tensor_copy(b2row, rows_ps[:, C:2 * C])

        # phase C: T1 = (K/N) L'^T;  ET = R^T T1 + rank1 + I
        T1_sb = []
        for cb in range(NCH):
            t1_ps = psacc.tile([128, C], f32, tag="acc")
            csl = slice(cb * 128, (cb + 1) * 128)
            for jb in range(NCH):
                nc.tensor.matmul(t1_ps, K_sb[jb][:, csl], lt(jb),
                                 start=(jb == 0), stop=(jb == NCH - 1))
            t = workp.tile([128, C], f16, tag=f"T1{cb}")
            if cb == 0:
                nc.scalar.copy(t, t1_ps)
            else:
                nc.vector.tensor_copy(t, t1_ps)
            T1_sb.append(t)
        ET_sb = []
        for ob in range(NCH):
            et_ps = psacc.tile([128, C], f32, tag="acc")
            osl = slice(ob * 128, (ob + 1) * 128)
            for cb in range(NCH):
                nc.tensor.matmul(et_ps, rcw(cb, osl), T1_sb[cb],
                                 start=(cb == 0), stop=False)
            nc.tensor.matmul(et_ps, aux[:, osl], a1row, start=False, stop=False)
            nc.tensor.matmul(et_ps, b2row[:, osl], aux[:, C:2 * C],
                             start=False, stop=False)
            # += I_block via matmul (keeps DVE off the critical chain)
            nc.tensor.matmul(et_ps, ident[:, 0, 0:128], ident[:, ob, :],
                             start=False, stop=True)
            t = workp.tile([128, C], f16, tag=f"ET{ob}")
            if ob == 0:
                nc.scalar.copy(t, et_ps)
            else:
                nc.vector.tensor_copy(t, et_ps)
            ET_sb.append(t)

        # phase D: d = L'(K/N)ptb + c1 a1 + c2n wgb + W_b
        col_ps = pscol.tile([128, 12], f32, tag="cols")
        for cb in range(NCH):
            csl = slice(cb * 128, (cb + 1) * 128)
            for jb in range(NCH):
                nc.tensor.matmul(col_ps[:, cb:cb + 1], K_sb[jb][:, csl],
                                 ptbc(jb),
                                 start=(jb == 0), stop=(jb == NCH - 1))
        # c2n = (sxj.ptb)/N via the sxj/N column of K_sb (1-partition out)
        for jb in range(NCH):
            nc.tensor.matmul(col_ps[0:1, 6:7], sxn(jb), ptbc(jb),
                             start=(jb == 0), stop=(jb == NCH - 1))
        kpc = rowsp.tile([128, NCH], f16, tag="kpc")
        nc.scalar.copy(kpc, col_ps[:, 0:NCH])
        c2cell = rowsp.tile([1, 1], f16, tag="c2cell")
        nc.vector.tensor_copy(c2cell, col_ps[0:1, 6:7])
        for ob in range(NCH):
            osl = slice(ob * 128, (ob + 1) * 128)
            for cb in range(NCH):
                nc.tensor.matmul(col_ps[:, 2 + ob:3 + ob], lt(cb, osl),
                                 kpc[:, cb:cb + 1],
                                 start=(cb == 0), stop=False)
            nc.tensor.matmul(col_ps[:, 2 + ob:3 + ob], a1row[:, osl],
                             aux[:, 2 * C:2 * C + 1], start=False, stop=False)
            nc.tensor.matmul(col_ps[:, 2 + ob:3 + ob],
                             aux[:, C + ob * 128:C + (ob + 1) * 128],
                             c2cell, start=False, stop=True)
        dcol = rowsp.tile([128, NCH], f32, tag="dcol")
        nc.vector.tensor_add(dcol, col_ps[:, 2:2 + NCH], sm[:, :, 2])
        hp_ctx.__exit__(None, None, None)

        # ---- phase S: analytic BN stats ------------------------------
        K2_ps = [psg8.tile([128, C + 8], f32, tag="g8", name=f"K2_ps{m}")
                 for m in range(NCH)]
        gram(K2_ps, xiw)
        K2_sb = []
        gram_copy(K2_ps, K2_sb, "K2")

        def sxin(k):  # sxi/N column (f16)
            return K2_sb[k][:, C:C + 1]

        # QT[j, c] = (Kxi/N A^T)[j, c]; P = QT .* ET; diag = sum_j P
        P_sb = []
        for jb in range(NCH):
            qt_ps = psacc.tile([128, C], f32, tag="acc")
            jsl = slice(jb * 128, (jb + 1) * 128)
            for kb in range(NCH):
                nc.tensor.matmul(qt_ps, K2_sb[kb][:, jsl], ET_sb[kb],
                                 start=(kb == 0), stop=(kb == NCH - 1))
            p = workp.tile([128, C], f16, tag=f"P{jb}")
            nc.vector.tensor_mul(p, qt_ps, ET_sb[jb])
            P_sb.append(p)
        # col_ps cols 8-9: diag(A Kxi/N A^T); cols 10-11: asx = A sxi/N
        for cb in range(NCH):
            csl = slice(cb * 128, (cb + 1) * 128)
            for jb in range(NCH):
                nc.tensor.matmul(col_ps[:, 8 + cb:9 + cb], P_sb[jb][:, csl],
                                 ones1, start=(jb == 0), stop=(jb == NCH - 1))
            for kb in range(NCH):
                nc.tensor.matmul(col_ps[:, 10 + cb:11 + cb],
                                 ET_sb[kb][:, csl], sxin(kb),
                                 start=(kb == 0), stop=(kb == NCH - 1))
        # spack = [S1n_0 | S1n_1 | S2n_0 | S2n_1]  (per-core mean/meansq)
        # diag/asx read straight from PSUM cols (no staging copy)
        spack = rowsp.tile([128, 4], f32, tag="spack")
        tmp2 = rowsp.tile([128, NCH], f32, tag="tmp2")
        for cb in range(NCH):
            dc = dcol[:, cb:cb + 1]
            asxc = col_ps[:, 10 + cb:11 + cb]
            nc.vector.tensor_scalar(
                out=spack[:, cb:cb + 1], in0=asxc, scalar1=dc, scalar2=0.0,
                op0=mybir.AluOpType.add, op1=mybir.AluOpType.add)
            nc.vector.tensor_scalar(
                out=tmp2[:, cb:cb + 1], in0=asxc, scalar1=2.0, scalar2=dc,
                op0=mybir.AluOpType.mult, op1=mybir.AluOpType.add)
            nc.vector.tensor_scalar(
                out=spack[:, 2 + cb:3 + cb], in0=tmp2[:, cb:cb + 1],
                scalar1=dc, scalar2=col_ps[:, 8 + cb:9 + cb],
                op0=mybir.AluOpType.mult, op1=mybir.AluOpType.add)

        # ---- ONE ReduceScatter: input = own stats tiled 8x, so every --
        # ---- core's scattered block is already the full global sum ----
        cc_in = dramp.tile([NCORES * 128, 4], f32, tag="cc_in", name="cc_in")
        cc_out = dramp.tile([128, 4], f32, tag="cc_out", name="cc_out")
        nc.sync.dma_start(
            out=cc_in.rearrange("(r p) f -> p r f", p=128),
            in_=spack.unsqueeze(1).broadcast_to([128, NCORES, 4]))
        if skip_cc:
            nc.sync.dma_start(out=cc_out, in_=cc_in[0:128, :])
        else:
            nc.gpsimd.collective_compute(
                "ReduceScatter",
                mybir.AluOpType.add,
                replica_groups=[list(range(NCORES))],
                ins=[cc_in.opt()],
                outs=[cc_out.opt()],
            )
        ssum = rowsp.tile([128, 4], f32, tag="ssum")
        nc.sync.dma_start(out=ssum, in_=cc_out)

        # ---- phase Z: z = A xi (+d via ACT) -> z_t (f16) --------------
        z_t = datap.tile([128, NCH, N], f16, tag="z")
        NZT = 8
        for tp in range(NZT):
            tsl = slice(tp * 512, (tp + 1) * 512)
            for j in range(NCH):
                jsl = slice(j * 128, (j + 1) * 128)
                z_ps = psz.tile([128, 512], f32, tag="zt")
                for k in range(NCH):
                    nc.tensor.matmul(
                        z_ps, ET_sb[k][:, jsl], xi_t[:, k, tsl],
                        start=(k == 0), stop=(k == NCH - 1))
                nc.scalar.activation(
                    out=z_t[:, j, tsl], in_=z_ps,
                    func=mybir.ActivationFunctionType.Identity,
                    bias=dcol[:, j:j + 1], scale=1.0)

        # ---- affine constants: a = gamma*rsqrt(var+eps); b = mean*a-beta
        # ssum = (8*mean | 8*meansq) per chunk; fold the /8 into the ops.
        t2 = rowsp.tile([128, NCH], f32, tag="t2")
        nc.vector.tensor_mul(t2, ssum[:, 0:2], ssum[:, 0:2])
        v8 = rowsp.tile([128, NCH], f32, tag="v8")
        nc.vector.scalar_tensor_tensor(
            out=v8, in0=t2, scalar=-1.0 / NCORES, in1=ssum[:, 2:4],
            op0=mybir.AluOpType.mult, op1=mybir.AluOpType.add)
        scol = rowsp.tile([128, NCH], f32, tag="scol")
        nc.scalar.activation(
            out=scol, in_=v8, func=mybir.ActivationFunctionType.Sqrt,
            bias=eps, scale=1.0 / NCORES)
        acols = rowsp.tile([128, NCH], f32, tag="acols")
        nc.vector.reciprocal(out=acols, in_=scol)
        nc.vector.tensor_mul(acols, acols, sm[:, :, 0])
        bcols = rowsp.tile([128, NCH], f32, tag="bcols")
        nc.vector.scalar_tensor_tensor(
            out=bcols, in0=ssum[:, 0:2], scalar=1.0 / NCORES, in1=acols,
            op0=mybir.AluOpType.mult, op1=mybir.AluOpType.mult)
        nc.vector.scalar_tensor_tensor(
            out=bcols, in0=bcols, scalar=1.0, in1=sm[:, :, 1],
            op0=mybir.AluOpType.mult, op1=mybir.AluOpType.subtract)

        # ---- apply out = a*z - b on DVE (f16 4x mode), staircased -----
        PIECES = [(0, 0, 1024), (1, 0, 1024), (0, 1024, 1536),
                  (1, 1024, 1536), (0, 2560, 1536), (1, 2560, 1536)]
        for idx, (j, off, width) in enumerate(PIECES):
            qsl = slice(off, off + width)
            o16 = outp.tile([128, width], f16, tag=f"o16_{idx}")
            nc.vector.tensor_scalar(
                out=o16, in0=z_t[:, j, qsl],
                scalar1=acols[:, j:j + 1], scalar2=bcols[:, j:j + 1],
                op0=mybir.AluOpType.mult, op1=mybir.AluOpType.subtract)
            eng = nc.sync if idx % 2 == 0 else nc.scalar
            eng.dma_start(out=out_d[j * 128:(j + 1) * 128, qsl], in_=o16)


_NC_CACHE: dict = {}


def _get_nc():
    if "nc" not in _NC_CACHE:
        nc = bacc.Bacc(
            "TRN2",
            target_bir_lowering=False,
            debug=False,
            enable_asserts=True,
            num_devices=NCORES,
        )
        build_kernel(nc)
        nc.compile()
        _NC_CACHE["nc"] = nc
    return _NC_CACHE["nc"]


def _make_in_maps(inputs: dict) -> list[dict]:
    xi = np.asarray(inputs["xi"], np.float32).reshape(B, C, N)
    xj = np.asarray(inputs["xj"], np.float32).reshape(B, C, N)
    g_w = np.asarray(inputs["g_w"], np.float32)
    g_b = np.asarray(inputs["g_b"], np.float32)
    t_w = np.asarray(inputs["theta_w"], np.float32)
    t_b = np.asarray(inputs["theta_b"], np.float32)
    p_w = np.asarray(inputs["phi_w"], np.float32)
    p_b = np.asarray(inputs["phi_b"], np.float32)
    W_w = np.asarray(inputs["W_w"], np.float32)
    W_b = np.asarray(inputs["W_b"], np.float32)
    gam = np.asarray(inputs["bn_gamma"], np.float32)
    bet = np.asarray(inputs["bn_beta"], np.float32)

    def chunked(a):  # [256, F] -> [128, 2, F]
        return np.ascontiguousarray(a.reshape(2, 128, -1).transpose(1, 0, 2))

    # host-folded weight products (constant folding, fp32)
    Lp = W_w @ g_w                      # L' = W G   (device uses K/N)
    R = p_w.T @ t_w                     # R = P^T T
    wgb = W_w @ g_b
    b1 = t_w.T @ p_b
    ptb = p_w.T @ t_b
    c1 = float(p_b @ t_b)

    wm = np.zeros((128, NCH, WM_F), np.float16)
    wm[:, :, 0:C] = chunked(Lp.T)
    wm[:, :, C:2 * C] = chunked(R)
    wm[:, :, 2 * C] = ptb.reshape(2, 128).T
    aux = np.zeros((1, 2 * C + 8), np.float16)
    aux[0, 0:C] = b1.astype(np.float16)
    aux[0, C:2 * C] = wgb.astype(np.float16)
    aux[0, 2 * C] = np.float16(c1)
    sm = np.zeros((128, NCH, 3), np.float32)
    sm[:, :, 0] = gam.reshape(2, 128).T
    sm[:, :, 1] = bet.reshape(2, 128).T
    sm[:, :, 2] = W_b.reshape(2, 128).T

    def sw8(X):  # [C, N] -> fp8 SwInterleave weights [128, 16, 2, 129, 2]
        # w8[p, t, m, j, i] = X[m*128 + 127 - j, t*256 + i*128 + p]
        # (j < 128); w8[p, t, m, 128, i] = 1.0 (ones rhs column)
        Xr = X.reshape(2, 128, NT2, 2, 128)      # [m, c, t, i, p]
        w = Xr[:, ::-1].transpose(4, 2, 0, 1, 3)  # [p, t, m, j, i]
        o = np.ones((128, NT2, 2, 129, 2), np.float32)
        o[:, :, :, 0:128, :] = w
        return o.astype(ml_dtypes.float8_e4m3)

    in_maps = []
    for b in range(B):
        xib = chunked(xi[b]).astype(np.float16)      # [128,2,4096]
        in_maps.append({
            "xjw8": sw8(xj[b]), "xiw8": sw8(xi[b]), "xi": xib, "wm": wm,
            "aux": aux, "sm": sm,
        })
    return in_maps


def kernel(**inputs) -> np.ndarray:
    nc = _get_nc()
    in_maps = _make_in_maps(inputs)
    last_err = None
    for attempt in range(3):
        try:
            res = bass_utils.run_bass_kernel_spmd(
                nc, in_maps, core_ids=list(range(NCORES)),
            )
            break
        except Exception as e:  # transient device wedge: back off and retry
            last_err = e
            import time as _time
            _time.sleep(4.0 * (attempt + 1))
            try:
                import jax
                import jax.extend.backend as _jeb
                jax.clear_caches()
                _jeb.clear_backends()
            except Exception:
                pass
    else:
        raise last_err
    out = np.stack([res.results[c]["out"] for c in range(NCORES)])
    return np.ascontiguousarray(out.reshape(B, C, 64, 64).astype(np.float32))


if __name__ == "__main__":
    rng = np.random.default_rng(0)
    fake = {
        "xi": rng.standard_normal((B, C, 64, 64)).astype(np.float32),
        "xj": rng.standard_normal((B, C, 64, 64)).astype(np.float32),
        "g_w": (rng.standard_normal((C, C)) / 16).astype(np.float32),
        "g_b": (rng.standard_normal((C,)) / 16).astype(np.float32),
        "theta_w": (rng.standard_normal((C, C)) / 16).astype(np.float32),
        "theta_b": (rng.standard_normal((C,)) / 16).astype(np.float32),
        "phi_w": (rng.standard_normal((C, C)) / 16).astype(np.float32),
        "phi_b": (rng.standard_normal((C,)) / 16).astype(np.float32),
        "W_w": (rng.standard_normal((C, C)) / 16).astype(np.float32),
        "W_b": (rng.standard_normal((C,)) / 16).astype(np.float32),
        "bn_gamma": np.ones((C,), np.float32),
        "bn_beta": np.zeros((C,), np.float32),
    }
    out = kernel(**fake)
    print("out", out.shape, out.dtype, float(np.abs(out).mean()))
